# revision 1
# baseline (speedup 1.0000x reference)
"""Trainium2 Bass kernel for nn_DifferentiableRiskBudgeting.

Solves, per batch sample b:
    min_w  w' S_b w - beta_b' w + lam1*||w||_1 + lam2*||w - w_prev||^2
    s.t.   sum w = 1, 0 <= w <= MAX_W
then clamps + renormalizes — matching the reference's converged
projected-gradient solution (the QP is strongly convex so the fixed
point is unique; we reach it with FISTA + a warm-started Newton
projection instead of the reference's 250 plain PGD iterations with a
30-step bisection per projection).

Sharding: pure data parallel, batch 512 = 64 samples per core on 8
cores. Each core keeps its sigma shard resident in SBUF (16.8MB) and
runs, per sample:
  - 10 power iterations (sum-of-squares normalization; the Rayleigh
    quotient at the end is scale-free) for lambda_max
  - FISTA (60 iters), per-sample step 1/L, L = 1.15*(2*lmax + 2*lam2)
  - capped-simplex projection via damped Newton on the piecewise-linear
    sum equation (slope = count+1, fused as (c1+1)-c2 so it is always
    >= 1), warm-started from the previous iteration's tau (validated:
    worst projection error 1.5e-8 over the whole trajectory)

Matvec mapping: per (sample, j-half, i-half) one PE matmul with the
sigma chunk [128,128] as the stationary operand (sigma is symmetric so
row-major storage doubles as the transposed operand) and the sample's
z-column [128,1] as the moving operand, accumulating column-major
y[i, sample] in PSUM (output base partition must be 0). DVE 32x32
block transposes read PSUM directly and produce the sample-major
y-copy the projection wants. Samples run in two groups of 32 so the
DVE/ACT projection of one group overlaps the PE matvec of the other.

Raw bass (no Tile): this container's walrus build only allows ~2 sync
commands per instruction, which Tile's scheduler exceeds at every
cross-engine join. With explicit semaphores every wait is a standalone
single-wait instruction; all semaphore values are static because the
schedule is fully unrolled. Same-engine dependent ops also need a
producer-inc + consumer-wait pair (engine pipelines do not interlock),
with ordering transitive through any later same-engine inc.
"""

import math
import numpy as np
from contextlib import ExitStack

import concourse.bass as bass
from concourse import mybir
from concourse.bass_utils import run_bass_kernel_spmd

F32 = mybir.dt.float32
ALU = mybir.AluOpType
ACTF = mybir.ActivationFunctionType

B, P = 512, 256
N_CORES = 8
NB = B // N_CORES            # samples per core
HALF = P // 128              # sigma row-halves (2)
GB = 32                      # pipeline group size
MAX_W = 0.1
EPS = 1e-8

NPOW = 10                    # power iterations
T_FISTA = 60                 # FISTA iterations
NEWTON = 5                   # Newton steps per projection
NEWTON0 = 8                  # first projection (cold tau)
SAFETY = 1.15                # L overestimation factor
SIG_DMA_BATCH = 4            # samples per sigma DMA

# set by the test harness; ignored by graders
TRACE = False
LAST_RESULT = None


def _emit(ctx, nc, sigma_d, beta_d, wprev_d, out_d, lam1, lam2, nb, npow, T):
    gb = min(GB, nb)
    ngrp = nb // gb
    assert ngrp * gb == nb and gb % 32 == 0

    def sbuf(name, shape):
        return ctx.enter_context(nc.sbuf_tensor(name, shape, F32))

    def psum(name):
        # full-bank tensors so PE writes and DVE reads of different
        # buffers can never share a PSUM bank (fatal on HW)
        return ctx.enter_context(nc.psum_tensor(name, [128, 512], F32))

    sems = {e: ctx.enter_context(nc.semaphore(f"s_{e}"))
            for e in ("pe", "dve", "act", "pool", "dma_bw", "dma_out")}
    for g in range(ngrp):
        sems[f"dma_sig{g}"] = ctx.enter_context(nc.semaphore(f"s_dma_sig{g}"))
    ENG = {"pe": nc.tensor, "dve": nc.vector, "act": nc.scalar,
           "pool": nc.gpsimd, "sync": nc.sync}
    ctr = {e: 0 for e in sems}
    last_wait = {e: {} for e in list(ENG)}

    def inc(ename, inst, n=1):
        ctr[ename] += n
        inst.then_inc(sems[ename], n)
        return ctr[ename]

    def wait(consumer, producer, value):
        if value is None or value <= 0:
            return
        lw = last_wait[consumer]
        if lw.get(producer, 0) >= value:
            return
        ENG[consumer].wait_ge(sems[producer], value)
        lw[producer] = value

    def dchain(inst):
        """Close a same-engine DVE dependency: inc on the producer, wait
        immediately after (ordering is transitive through this inc)."""
        t = inc("dve", inst)
        wait("dve", "dve", t)
        return t

    # ---------------- tensors
    ident = sbuf("ident", [128, 128])
    nbatch = SIG_DMA_BATCH
    nk = (nb + nbatch - 1) // nbatch
    sig = [sbuf(f"sig{k}", [128, nbatch * HALF * P]) for k in range(nk)]

    def sig_ap(b, hj, hi):
        k, m = divmod(b, nbatch)
        c0 = (m * HALF + hj) * P + hi * 128
        return sig[k][:, c0:c0 + 128]

    def gt(name, shape):
        return [sbuf(f"{name}{g}", shape) for g in range(ngrp)]

    z = gt("z", [gb, P])
    wA = gt("wA", [gb, P])
    wB = gt("wB", [gb, P])
    fv = gt("fv", [gb, P])
    beta_g = gt("beta", [gb, P])
    wprev_g = gt("wprev", [gb, P])
    v = gt("v", [gb, P])
    vc = gt("vc", [gb, P])
    t1 = gt("t1", [gb, P])
    dw = gt("dw", [gb, P])
    ysm = gt("ysm", [gb, P])
    outt = gt("outt", [gb, P])
    sa = gt("sa", [gb, P])       # ACT dummy outs
    sb = gt("sb", [gb, P])
    sc = gt("sc", [gb, P])       # DVE dummy outs
    sd = gt("sd", [gb, P])
    zT = [[sbuf(f"zT{g}_{h}", [128, gb]) for h in range(HALF)]
          for g in range(ngrp)]
    tiny_names = ("tau tneg tcneg s1 s2 c1 c2 phi cnt rc stp m2a dv ev "
                  "th m2 rcp num den rden lmax Lt sq onem onep rop ssum rs")
    TN = {}
    for name in tiny_names.split():
        TN[name] = gt(name, [gb, 1])

    ypsum = [[psum(f"y{g}_{p}") for p in range(2)] for g in range(ngrp)]
    ptpsum = [psum(f"pt{h}") for h in range(HALF)]

    # ---------------- events
    E_zready = {}        # g -> dve tick: z[g] ready for transpose
    E_ysm_done = {}      # (t, g) -> dve tick: y psum buffer consumed
    E_pt_free = 0        # act tick: previous pt banks consumed
    E_out_dve = {}

    # ---------------- preamble
    mz = nc.vector.memset(ident[:], 0.0)
    E_identz = inc("dve", mz)
    wait("pool", "dve", E_identz)
    af = nc.gpsimd.affine_select(
        out=ident[:], in_=ident[:], compare_op=ALU.not_equal, fill=1.0,
        base=0, pattern=[[-1, 128]], channel_multiplier=1)
    E_ident = inc("pool", af)

    kb_per_g = nk // ngrp
    for k in range(nk):
        kn = min(nbatch, nb - k * nbatch)
        srca = sigma_d[k * nbatch:k * nbatch + kn].rearrange(
            "b (h p) j -> p b h j", p=128)
        dst = sig[k][:].rearrange("p (b h j) -> p b h j", b=kn, h=HALF)
        d = nc.sync.dma_start(out=dst, in_=srca)
        d.then_inc(sems[f"dma_sig{k // kb_per_g}"], 16)
    E_sig_g = {g: 16 * kb_per_g for g in range(ngrp)}
    for g in range(ngrp):
        g0 = g * gb
        d = nc.sync.dma_start(out=beta_g[g][:], in_=beta_d[g0:g0 + gb, :])
        d.then_inc(sems["dma_bw"], 16)
        d = nc.sync.dma_start(out=wprev_g[g][:], in_=wprev_d[g0:g0 + gb, :])
        d.then_inc(sems["dma_bw"], 16)
    E_bw = 32 * ngrp

    for g in range(ngrp):
        nc.vector.memset(z[g][:], 1.0 / math.sqrt(P))
        nc.vector.memset(wA[g][:], 1.0 / P)
        m = nc.vector.memset(TN["tau"][g][:], 0.0)
        E_zready[g] = inc("dve", m)

    # ---------------- helpers
    def pe_transpose_and_matvec(t, g):
        nonlocal E_pt_free
        # transposes: z[g] -> zT (via pt psum banks), then the matvec
        wait("pe", "dve", E_zready[g])
        wait("pe", "act", E_pt_free)
        wait("pe", "pool", E_ident)
        for h in range(HALF):
            tr = nc.tensor.transpose(ptpsum[h][:, 0:gb],
                                     z[g][:, h * 128:(h + 1) * 128],
                                     ident[:gb, :gb])
            if h == HALF - 1:
                E_T = inc("pe", tr)
        # ACT copies pt -> zT
        wait("act", "pe", E_T)
        for h in range(HALF):
            cp = nc.scalar.copy(zT[g][h][:, :], ptpsum[h][:, 0:gb])
            if h == HALF - 1:
                E_zT = inc("act", cp)
        E_pt_free = E_zT
        # matvec into y psum buffer t%2
        yp = ypsum[g][t % 2]
        wait("pe", "act", E_zT)
        wait("pe", "dve", E_ysm_done.get((t - 2, g), 0))
        if t == 0:
            wait("pe", f"dma_sig{g}", E_sig_g[g])
        g0 = g * gb
        for bb in range(gb):
            b = g0 + bb
            for hi in range(HALF):
                for hj in range(HALF):
                    mm = nc.tensor.matmul(
                        yp[:, hi * gb + bb:hi * gb + bb + 1],
                        sig_ap(b, hj, hi),
                        zT[g][hj][:, bb:bb + 1],
                        start=(hj == 0), stop=(hj == HALF - 1))
        E_M = inc("pe", mm)
        return E_M

    def dve_ysm(t, g, E_M):
        # sample-major copy of y straight out of PSUM via 32x32 blocks
        yp = ypsum[g][t % 2]
        wait("dve", "pe", E_M)
        for hi in range(HALF):
            for q in range(4):
                tr = nc.vector.transpose(
                    ysm[g][0:32, hi * 128 + q * 32:hi * 128 + (q + 1) * 32],
                    yp[q * 32:(q + 1) * 32, hi * gb:hi * gb + 32])
        dchain(tr)

    # ---------------- power iterations
    for t in range(npow):
        for g in range(ngrp):
            E_M = pe_transpose_and_matvec(t, g)
            dve_ysm(t, g, E_M)
            i = nc.vector.scalar_tensor_tensor(sc[g][:], ysm[g][:], 1.0,
                                               ysm[g][:], ALU.mult, ALU.mult,
                                               accum_out=TN["m2"][g][:])
            dchain(i)
            i = nc.vector.reciprocal(TN["rcp"][g][:], TN["m2"][g][:])
            dchain(i)
            zi = nc.vector.tensor_scalar(z[g][:], ysm[g][:], TN["rcp"][g][:],
                                         None, ALU.mult)
            E_zready[g] = inc("dve", zi)

    # ---------------- Rayleigh quotient -> step sizes, FISTA coefficients
    t_ray = npow
    for g in range(ngrp):
        E_M = pe_transpose_and_matvec(t_ray, g)
        dve_ysm(t_ray, g, E_M)
        nc.vector.scalar_tensor_tensor(sc[g][:], z[g][:], 1.0, ysm[g][:],
                                       ALU.mult, ALU.mult,
                                       accum_out=TN["num"][g][:])
        i = nc.vector.scalar_tensor_tensor(sd[g][:], z[g][:], 1.0, z[g][:],
                                           ALU.mult, ALU.mult,
                                           accum_out=TN["den"][g][:])
        dchain(i)
        i = nc.vector.tensor_scalar(TN["den"][g][:], TN["den"][g][:], EPS,
                                    None, ALU.add)
        dchain(i)
        i = nc.vector.reciprocal(TN["rden"][g][:], TN["den"][g][:])
        dchain(i)
        i = nc.vector.tensor_tensor(TN["lmax"][g][:], TN["num"][g][:],
                                    TN["rden"][g][:], ALU.mult)
        dchain(i)
        i = nc.vector.tensor_scalar(TN["Lt"][g][:], TN["lmax"][g][:],
                                    2.0 * SAFETY, SAFETY * 2.0 * lam2,
                                    ALU.mult, ALU.add)
        dchain(i)
        i = nc.vector.reciprocal(TN["stp"][g][:], TN["Lt"][g][:])
        dchain(i)
        nc.vector.tensor_scalar(TN["m2a"][g][:], TN["stp"][g][:], -2.0, None,
                                ALU.mult)
        dvi = nc.vector.tensor_scalar(TN["dv"][g][:], TN["stp"][g][:],
                                      2.0 * lam2, None, ALU.mult)
        E_dv = dchain(dvi)
        nc.vector.tensor_scalar(TN["ev"][g][:], TN["dv"][g][:], -1.0, 1.0,
                                ALU.mult, ALU.add)
        # theta = (1 - sqrt(q)) / (1 + sqrt(q)), q = 2*lam2*step
        wait("act", "dve", E_dv)
        sq = nc.scalar.activation(TN["sq"][g][:], TN["dv"][g][:], ACTF.Sqrt)
        E_sq = inc("act", sq)
        wait("dve", "act", E_sq)
        nc.vector.tensor_scalar(TN["onem"][g][:], TN["sq"][g][:], -1.0, 1.0,
                                ALU.mult, ALU.add)
        i = nc.vector.tensor_scalar(TN["onep"][g][:], TN["sq"][g][:], 1.0,
                                    None, ALU.add)
        dchain(i)
        i = nc.vector.reciprocal(TN["rop"][g][:], TN["onep"][g][:])
        dchain(i)
        nc.vector.tensor_tensor(TN["th"][g][:], TN["onem"][g][:],
                                TN["rop"][g][:], ALU.mult)
        # fv = step*(beta - lam1) + dv*wprev
        wait("dve", "dma_bw", E_bw)
        i = nc.vector.tensor_scalar(fv[g][:], beta_g[g][:], lam1,
                                    TN["stp"][g][:], ALU.subtract, ALU.mult)
        dchain(i)
        nc.vector.scalar_tensor_tensor(fv[g][:], wprev_g[g][:],
                                       TN["dv"][g][:], fv[g][:],
                                       ALU.mult, ALU.add)
        zi = nc.vector.tensor_copy(z[g][:], wA[g][:])
        E_zready[g] = inc("dve", zi)

    # ---------------- FISTA
    for ti in range(T):
        t = npow + 1 + ti
        wold = wA if ti % 2 == 0 else wB
        wnew = wB if ti % 2 == 0 else wA
        for g in range(ngrp):
            E_M = pe_transpose_and_matvec(t, g)
            dve_ysm(t, g, E_M)
            # v = ev*z - 2*step*y + fv ; vc = v - MAX_W
            i = nc.vector.scalar_tensor_tensor(v[g][:], ysm[g][:],
                                               TN["m2a"][g][:], fv[g][:],
                                               ALU.mult, ALU.add)
            dchain(i)
            i = nc.vector.scalar_tensor_tensor(v[g][:], z[g][:],
                                               TN["ev"][g][:], v[g][:],
                                               ALU.mult, ALU.add)
            dchain(i)
            i = nc.vector.tensor_scalar(vc[g][:], v[g][:], MAX_W, None,
                                        ALU.subtract)
            dchain(i)
            ni = NEWTON0 if ti == 0 else NEWTON
            for k in range(ni):
                # sum relu(v-tau) = sum max(v,tau) - P*tau, so
                # phi = sum[max(v,tau) - max(v-c,tau)] - 1 needs no ACT and
                # no negated-tau biases. Slope = c1+1 (damped; capped count
                # c2 is ~0 in practice). All DVE, 3 streaming ops.
                nc.vector.tensor_scalar(sc[g][:], v[g][:], TN["tau"][g][:],
                                        None, ALU.max, ALU.add,
                                        accum_out=TN["s1"][g][:])
                nc.vector.tensor_scalar(sd[g][:], vc[g][:], TN["tau"][g][:],
                                        None, ALU.max, ALU.add,
                                        accum_out=TN["s2"][g][:])
                c1i = nc.vector.tensor_scalar(sa[g][:], v[g][:],
                                              TN["tau"][g][:], None,
                                              ALU.is_gt, ALU.add,
                                              accum_out=TN["c1"][g][:])
                dchain(c1i)
                nc.vector.scalar_tensor_tensor(TN["phi"][g][:],
                                               TN["s1"][g][:], 1.0,
                                               TN["s2"][g][:], ALU.subtract,
                                               ALU.subtract)
                ci = nc.vector.tensor_scalar(TN["cnt"][g][:], TN["c1"][g][:],
                                             1.0, None, ALU.add)
                dchain(ci)
                i = nc.vector.reciprocal(TN["rc"][g][:], TN["cnt"][g][:])
                dchain(i)
                ta = nc.vector.scalar_tensor_tensor(TN["tau"][g][:],
                                                    TN["phi"][g][:],
                                                    TN["rc"][g][:],
                                                    TN["tau"][g][:],
                                                    ALU.mult, ALU.add)
                dchain(ta)
            # w_new = clip(v - tau, 0, MAX_W); dw = w_new - w_old
            i = nc.vector.tensor_scalar(t1[g][:], v[g][:], TN["tau"][g][:],
                                        0.0, ALU.subtract, ALU.max)
            dchain(i)
            wi = nc.vector.tensor_scalar(wnew[g][:], t1[g][:], MAX_W, None,
                                         ALU.min)
            dchain(wi)
            if ti < T - 1:
                i = nc.vector.scalar_tensor_tensor(dw[g][:], t1[g][:], MAX_W,
                                                   wold[g][:], ALU.min,
                                                   ALU.subtract)
                dchain(i)
                # z = w_new + th*dw
                zi = nc.vector.scalar_tensor_tensor(z[g][:], dw[g][:],
                                                    TN["th"][g][:],
                                                    wnew[g][:], ALU.mult,
                                                    ALU.add)
                E_zready[g] = inc("dve", zi)
            else:
                # renormalize and stage the output
                i = nc.vector.tensor_scalar(sd[g][:], wnew[g][:], 0.0, None,
                                            ALU.add, ALU.add,
                                            accum_out=TN["ssum"][g][:])
                dchain(i)
                i = nc.vector.tensor_scalar(TN["ssum"][g][:],
                                            TN["ssum"][g][:], EPS, None,
                                            ALU.add)
                dchain(i)
                i = nc.vector.reciprocal(TN["rs"][g][:], TN["ssum"][g][:])
                dchain(i)
                oi = nc.vector.tensor_scalar(outt[g][:], wnew[g][:],
                                             TN["rs"][g][:], None, ALU.mult)
                E_out_dve[g] = inc("dve", oi)

    # ---------------- store
    for g in range(ngrp):
        g0 = g * gb
        wait("sync", "dve", E_out_dve[g])
        d = nc.sync.dma_start(out=out_d[g0:g0 + gb, :], in_=outt[g][:])
        d.then_inc(sems["dma_out"], 16)
    nc.sync.wait_ge(sems["dma_out"], 16 * ngrp)


def build(lam1, lam2, nb=NB, npow=NPOW, T=T_FISTA):
    nc = bass.Bass("TRN2", target_bir_lowering=False, debug=False)
    sigma_d = nc.dram_tensor("sigma", [nb, P, P], F32, kind="ExternalInput")
    beta_d = nc.dram_tensor("beta", [nb, P], F32, kind="ExternalInput")
    wprev_d = nc.dram_tensor("w_prev", [nb, P], F32, kind="ExternalInput")
    out_d = nc.dram_tensor("out", [nb, P], F32, kind="ExternalOutput")
    with ExitStack() as ctx:
        _emit(ctx, nc, sigma_d.ap(), beta_d.ap(), wprev_d.ap(), out_d.ap(),
              lam1, lam2, nb, npow, T)
    return nc


def kernel(sigma, beta, w_prev, log_lambda1, log_lambda2):
    global LAST_RESULT
    sigma = np.ascontiguousarray(np.asarray(sigma, dtype=np.float32))
    beta = np.ascontiguousarray(np.asarray(beta, dtype=np.float32))
    w_prev = np.ascontiguousarray(np.asarray(w_prev, dtype=np.float32))
    lam1 = float(np.exp(np.float32(log_lambda1)))
    lam2 = float(np.exp(np.float32(log_lambda2)))

    nc = build(lam1, lam2)
    in_maps = []
    for c in range(N_CORES):
        s = slice(c * NB, (c + 1) * NB)
        in_maps.append({
            "sigma": sigma[s],
            "beta": beta[s],
            "w_prev": w_prev[s],
        })
    res = run_bass_kernel_spmd(nc, in_maps, list(range(N_CORES)), trace=TRACE)
    LAST_RESULT = res
    out = np.concatenate([res.results[c]["out"] for c in range(N_CORES)],
                         axis=0)
    return np.ascontiguousarray(out.astype(np.float32))



# revision 33
# speedup vs baseline: 9.5564x; 9.5564x over previous
"""Trainium2 Bass kernel for nn_DifferentiableRiskBudgeting.

Per batch sample b (data parallel, 64 samples per core on 8 cores):
    min_w  w' S_b w - beta_b' w + lam1*||w||_1 + lam2*||w - w_prev||^2
    s.t.   sum w = 1, 0 <= w <= MAX_W
then clamp + renormalize (matching the reference's converged 250-step
PGD fixed point; the QP is strongly convex so the fixed point is
unique).

Algorithm (numpy-validated, rel err ~2.8e-3 vs reference incl. fp16
iterate storage, gate 2e-2):
  - NPOW unnormalized power rounds z <- S z, then Rayleigh quotient
    R = (z3.z2)/(z2.z2) and per-sample step 1/(2*SAFETY*R + 2*lam2).
    FISTA tolerates steps up to 2/L so SAFETY=1.15 on the power
    underestimate is safe (validated worst-case underestimate 1.6x).
  - FISTA with per-sample momentum theta=(1-sqrt(q))/(1+sqrt(q)),
    q = 2*lam2*step, and the scaled-momentum state ws = theta*w so the
    critical-path z update is a single scalar_tensor_tensor:
        z = (1+theta)*w_new - ws;   ws = theta*w_new (off-path)
  - Simplex projection via ONE warm damped-Newton step per iteration
    (N0 cold steps on the first), ignoring the w<=MAX_W cap in the tau
    solve (cap binds for ~3 of 512 samples; the clip + final
    renormalize absorb it — validated):
        phi  = sum max(v,tau) - P*tau - 1
        cnt  = #(v > tau) + 1          (damped slope)
        tau += phi / cnt

Layouts: row [64, 256] (partition = sample, free = feature) for all
elementwise work, so per-sample scalars are per-partition APs and
tensor_scalar accumulations give full per-sample sums directly (walrus
requires matching partition ranges on elementwise operands and
PSUM access patterns starting at partition 0 — no cross-partition
folds are legal).  Column [128, 128] (partition = j within half hj,
free = hj*64 + b) for the PE matvec; sigma symmetric so row-major
chunks double as the transposed stationary operand.

Sigma DMA exploits symmetry: only rows 0:128 (A|B) and the C block are
loaded (75% of bytes); the B^T chunk is reconstructed on-device by PE
transposes + ACT/DVE copies hidden under the DMA (~38us at the modeled
360 GB/s).  Power rounds also pipeline per 8-sample batch under the
DMA, so the spectral estimate is free.

Iterate tensors are fp16 (DVE 4x mode on the clip/reduction passes);
the matvec path (sigma, zT, PSUM) stays fp32.  The y halves land in
two separate PSUM banks so a consumer read of half 0 never shares a
bank with the in-flight PE writes of half 1 (same-bank PE-write +
engine-read is fatal on HW).

Raw bass with explicit semaphores; waits are FUSED onto the consuming
instruction (1 wait + 1 inc <= walrus' ~2 sync commands per
instruction), extra waits standalone.  Same-engine dependent ops also
need inc+wait pairs (engine pipelines do not interlock); ordering is
transitive through any later same-engine inc.
"""

import math
import numpy as np
from contextlib import ExitStack

import concourse.bass as bass
from concourse import mybir
from concourse.bass_utils import run_bass_kernel_spmd

F32 = mybir.dt.float32
F16 = mybir.dt.float16
ALU = mybir.AluOpType
ACTF = mybir.ActivationFunctionType

B, P = 512, 256
N_CORES = 8
NB = B // N_CORES            # samples per core (64)
H = 2                        # feature halves
MAX_W = 0.1
EPS = 1e-8

NPOW = 3                     # unnormalized power rounds
SAFETY = 1.15
T_FISTA = 22
N0 = 5                       # Newton steps, first iteration
SIG_DMA_BATCH = 4            # samples per sigma DMA
POW_BATCH = 8                # samples per pipelined power batch

TRACE = False
LAST_RESULT = None


def _emit(ctx, nc, sigma_d, beta_d, wprev_d, out_d, lam1, lam2,
          T=None, npow=None):
    nb = NB
    T = T_FISTA if T is None else T
    npow_ = NPOW if npow is None else npow

    def sbuf(name, shape, dt=F32):
        return ctx.enter_context(nc.sbuf_tensor(name, shape, dt))

    def psum(name):
        return ctx.enter_context(nc.psum_tensor(name, [128, 512], F32))

    ENG = {"pe": nc.tensor, "dve": nc.vector, "act": nc.scalar,
           "pool": nc.gpsimd, "sync": nc.sync}
    sems = {e: ctx.enter_context(nc.semaphore(f"s_{e}"))
            for e in ("pe", "dve", "act", "pool", "dma_bw", "dma_out")}
    npb = NB // POW_BATCH
    for g in range(npb):
        sems[f"dma_sig{g}"] = ctx.enter_context(nc.semaphore(f"s_dsig{g}"))
    ctr = {e: 0 for e in sems}
    last_wait = {e: {} for e in list(ENG)}

    def eop(eng, emit, waits=(), inc=True):
        """Emit an op on `eng` with fused sync (see module docstring)."""
        lw = last_wait[eng]
        pend = []
        for s, v in waits:
            if v and v > 0 and lw.get(s, 0) < v:
                pend.append((s, v))
                lw[s] = v
        for s, v in pend[1:]:
            ENG[eng].wait_ge(sems[s], v)
        inst = emit()
        if pend:
            s, v = pend[0]
            inst.wait_op(sems[s], v, "sem-ge")
        if inc:
            ctr[eng] += 1
            inst.then_inc(sems[eng], 1)
            return ctr[eng]
        return None

    # ------------------------------------------------------------- tensors
    nbatch = SIG_DMA_BATCH
    nk = nb // nbatch
    sig = [sbuf(f"sig{k}", [128, nbatch * H * P]) for k in range(nk)]

    def sig_ap(b, hj, hi):
        k, m = divmod(b, nbatch)
        c0 = (m * H + hj) * P + hi * 128
        return sig[k][:, c0:c0 + 128]

    ident = sbuf("ident", [128, 128])
    ones = sbuf("ones", [128, 1])
    zT = sbuf("zT", [128, 128])          # col: [j-in-half, hj*64+b], f32
    ycol = sbuf("ycol", [128, 128])      # col: [i-in-half, hi*64+b], f32
    pq = sbuf("pq", [128, 128])
    qq = sbuf("qq", [128, 128])
    zrow = sbuf("zrow", [64, 256])       # row: [b, j]
    u = sbuf("u", [64, 256])
    fv = sbuf("fv", [64, 256])
    fvw = sbuf("fvw", [64, 256])
    vrow = sbuf("vrow", [64, 256], F16)
    t1 = sbuf("t1", [64, 256], F16)
    wA = sbuf("wA", [64, 256], F16)
    wB = sbuf("wB", [64, 256], F16)
    ws = sbuf("ws", [64, 256], F16)
    brow = sbuf("brow", [64, 256])
    wprow = sbuf("wprow", [64, 256])
    outt = sbuf("outt", [64, 256])
    dumA = sbuf("dumA", [64, 256], F16)
    dumB = sbuf("dumB", [64, 256], F16)
    SCT = sbuf("SCT", [64, 8])           # cols: stp, ev, m2a, dv, th, th1
    SCX = sbuf("SCX", [1, 1024])         # [1,64] scratch slices
    tau = sbuf("tau", [64, 1])
    S1a = sbuf("S1a", [64, 1])
    cnta = sbuf("cnta", [64, 1])
    X64 = sbuf("X64", [64, 1])
    G64 = sbuf("G64", [64, 1])
    rc = sbuf("rc", [64, 1])
    ssum = sbuf("ssum", [64, 1])
    rs = sbuf("rs", [64, 1])

    PBzt = psum("PBzt")
    PBy = psum("PBy")
    PBy1 = psum("PBy1")      # hi=1 matvec bank (bank-isolated from PBy)
    PByr = psum("PByr")
    PBr = psum("PBr")

    STP = SCT[:, 0:1]
    EV = SCT[:, 1:2]
    M2A = SCT[:, 2:3]
    DV = SCT[:, 3:4]
    TH = SCT[:, 4:5]
    TH1 = SCT[:, 5:6]

    def scx(k):
        return SCX[0:1, 64 * k:64 * (k + 1)]

    # --------------------------------------------------------------- DMAs
    # sigma symmetric: load A|B rows + C block; B^T rebuilt on-device.
    kb_per_g = nk // npb
    for k in range(nk):
        ks = sigma_d[k * nbatch:(k + 1) * nbatch]
        dst4 = sig[k][:].rearrange("p (b h j) -> p b h j", b=nbatch, h=H)
        d = nc.sync.dma_start(
            out=dst4[:, :, 0, :],
            in_=ks[:, 0:128, :].rearrange("b p j -> p b j"))
        d.then_inc(sems[f"dma_sig{k // kb_per_g}"], 16)
        d = nc.sync.dma_start(
            out=dst4[:, :, 1, 128:256],
            in_=ks[:, 128:256, 128:256].rearrange("b p j -> p b j"))
        d.then_inc(sems[f"dma_sig{k // kb_per_g}"], 16)
    E_sig_g = 32 * kb_per_g
    d = nc.sync.dma_start(out=brow[:], in_=beta_d)
    d.then_inc(sems["dma_bw"], 16)
    d = nc.sync.dma_start(out=wprow[:], in_=wprev_d)
    d.then_inc(sems["dma_bw"], 16)
    E_bw = 32

    # ------------------------------------------------------------ preamble
    eop("dve", lambda: nc.vector.memset(ident[:], 0.0))
    eop("dve", lambda: nc.vector.memset(ones[:], 1.0))
    E_zT0 = eop("dve", lambda: nc.vector.memset(zT[:], 1.0 / math.sqrt(P)))
    E_ident = eop("pool", lambda: nc.gpsimd.affine_select(
        out=ident[:], in_=ident[:], compare_op=ALU.not_equal, fill=1.0,
        base=0, pattern=[[-1, 128]], channel_multiplier=1),
        waits=[("dve", E_zT0)])

    def matvec(gate_waits, b_range=range(nb), tick_each_hi=False):
        """Column-space matvec: y-half hi of sample b -> (PBy|PBy1)[:, b].

        Returns (tick_hi0, tick_all) pe ticks."""
        t_hi0 = None
        first = True
        for hi in range(H):
            dst = PBy if hi == 0 else PBy1
            for b in b_range:
                for hj in range(H):
                    def mk(b=b, hj=hj, hi=hi, dst=dst):
                        return nc.tensor.matmul(
                            dst[:, b:b + 1], sig_ap(b, hj, hi),
                            zT[:, hj * 64 + b:hj * 64 + b + 1],
                            start=(hj == 0), stop=(hj == H - 1))
                    is_last = (b == b_range[-1] and hj == H - 1)
                    if first:
                        tick = eop("pe", mk, waits=gate_waits)
                        first = False
                    elif is_last and (hi == H - 1 or tick_each_hi):
                        tick = eop("pe", mk)
                    else:
                        mk()
            if hi == 0:
                t_hi0 = tick if tick_each_hi else None
        return t_hi0, tick

    # ------------------------------- B^T reconstruction + power (pipelined)
    E_pmm_last = 0
    slot_last = {0: 0, 1: 0}          # last B-copy tick per slot (act/dve)
    for pb in range(npb):
        b0 = pb * POW_BATCH
        for bb in range(POW_BATCH):
            b = b0 + bb
            s = bb % 2
            bank = PBzt if s == 0 else PBr
            ceng = "act" if s == 0 else "dve"

            def tr(b=b, bank=bank):
                return nc.tensor.transpose(bank[:, 0:128],
                                           sig_ap(b, 0, 1), ident[:])
            E_tr = eop("pe", tr, waits=[
                (f"dma_sig{pb}", E_sig_g), ("pool", E_ident),
                (ceng, slot_last[s])])

            def cp(b=b, bank=bank, ceng=ceng):
                op = (nc.scalar.copy if ceng == "act"
                      else nc.vector.tensor_copy)
                return op(sig_ap(b, 1, 0), bank[:, 0:128])
            slot_last[s] = eop(ceng, cp, waits=[("pe", E_tr)])
        E_bt_act, E_bt_dve = slot_last[0], slot_last[1]

        E_cp = 0
        for r in range(npow_):
            w = [("pool", E_ident), ("dve", max(E_zT0, E_bt_dve)),
                 ("act", max(E_cp, E_bt_act))]
            _, E_mm = matvec(w, b_range=range(b0, b0 + POW_BATCH))
            if r < npow_ - 1:
                def cp0(b0=b0):
                    return nc.scalar.copy(
                        zT[:].rearrange("p (h b) -> p h b", h=H)[
                            :, 0, b0:b0 + POW_BATCH],
                        PBy[:, b0:b0 + POW_BATCH])

                def cp1(b0=b0):
                    return nc.scalar.copy(
                        zT[:].rearrange("p (h b) -> p h b", h=H)[
                            :, 1, b0:b0 + POW_BATCH],
                        PBy1[:, b0:b0 + POW_BATCH])
                eop("act", cp0, waits=[("pe", E_mm)])
                E_cp = eop("act", cp1)
            else:
                E_pmm_last = E_mm

    # --------------------------------------------------------- Rayleigh
    eop("dve", lambda: nc.vector.tensor_tensor(
        pq[:, 0:64], PBy[:, 0:64], zT[:, 0:64], ALU.mult),
        waits=[("pe", E_pmm_last)])
    E_pq = eop("dve", lambda: nc.vector.tensor_tensor(
        pq[:, 64:128], PBy1[:, 0:64], zT[:, 64:128], ALU.mult))
    E_qq = eop("dve", lambda: nc.vector.tensor_tensor(
        qq[:], zT[:], zT[:], ALU.mult), waits=[("dve", E_pq)])
    eop("pe", lambda: nc.tensor.matmul(
        PBr[0:1, 0:128], ones[:], pq[:], start=True, stop=True),
        waits=[("dve", E_qq)])
    E_red = eop("pe", lambda: nc.tensor.matmul(
        PBr[0:1, 128:256], ones[:], qq[:], start=True, stop=True))

    # t=0 state init (zT reused: re-memset after qq consumed it)
    E_zTi = eop("dve", lambda: nc.vector.memset(zT[:], 1.0 / P),
                waits=[("dve", E_qq)])
    E_zri = eop("dve", lambda: nc.vector.memset(zrow[:], 1.0 / P))
    E_wAi = eop("dve", lambda: nc.vector.memset(wA[:], 1.0 / P))
    E_tau = eop("dve", lambda: nc.vector.memset(tau[:], 0.0))

    # t=0 matvec (emitted early; PE works while DVE does the scalar chain)
    E_mm0_h0, E_mm0 = matvec([("dve", E_zTi)], tick_each_hi=True)

    # ------------------------------------------------------ scalar chain
    TS = nc.vector.tensor_scalar
    TT = nc.vector.tensor_tensor
    STT = nc.vector.scalar_tensor_tensor
    num, den, rden, R, L = scx(0), scx(1), scx(2), scx(3), scx(4)
    sq, onep, rop, onem = scx(5), scx(6), scx(7), scx(8)
    stp_r, ev_r, m2a_r, dv_r, th_r, th1_r = (scx(9), scx(10), scx(11),
                                             scx(12), scx(13), scx(14))

    prs = SCX[0:1, 768:1024]   # SBUF copy of the PE-reduce results
    e = eop("dve", lambda: nc.vector.tensor_copy(prs, PBr[0:1, 0:256]),
            waits=[("pe", E_red)])
    e = eop("dve", lambda: TT(num, SCX[0:1, 768:832], SCX[0:1, 832:896],
                              ALU.add), waits=[("dve", e)])
    e = eop("dve", lambda: TT(den, SCX[0:1, 896:960], SCX[0:1, 960:1024],
                              ALU.add), waits=[("dve", e)])
    e = eop("dve", lambda: TS(den, den, EPS, None, ALU.add),
            waits=[("dve", e)])
    e = eop("dve", lambda: nc.vector.reciprocal(rden, den),
            waits=[("dve", e)])
    e = eop("dve", lambda: TT(R, num, rden, ALU.mult), waits=[("dve", e)])
    e = eop("dve", lambda: TS(L, R, 2.0 * SAFETY, 2.0 * lam2 + 1e-6,
                              ALU.mult, ALU.add), waits=[("dve", e)])
    e = eop("dve", lambda: nc.vector.reciprocal(stp_r, L),
            waits=[("dve", e)])
    e = eop("dve", lambda: TS(ev_r, stp_r, -2.0 * lam2, 1.0, ALU.mult,
                              ALU.add), waits=[("dve", e)])
    e = eop("dve", lambda: TS(m2a_r, stp_r, -2.0, None, ALU.mult),
            waits=[("dve", e)])
    E_dv = eop("dve", lambda: TS(dv_r, stp_r, 2.0 * lam2, None, ALU.mult),
               waits=[("dve", e)])
    E_sq = eop("act", lambda: nc.scalar.activation(sq, dv_r, ACTF.Sqrt),
               waits=[("dve", E_dv)])
    e = eop("dve", lambda: TS(onep, sq, 1.0, None, ALU.add),
            waits=[("act", E_sq)])
    e = eop("dve", lambda: nc.vector.reciprocal(rop, onep),
            waits=[("dve", e)])
    e = eop("dve", lambda: TS(onem, sq, -1.0, 1.0, ALU.mult, ALU.add),
            waits=[("dve", e)])
    e = eop("dve", lambda: TT(th_r, onem, rop, ALU.mult),
            waits=[("dve", e)])
    E_scp = eop("dve", lambda: TS(th1_r, th_r, 1.0, None, ALU.add),
                waits=[("dve", e)])
    # transpose each [1,64] scalar row -> [64,1]; land in PBzt cols 0..5
    scalar_rows = [stp_r, ev_r, m2a_r, dv_r, th_r, th1_r]
    for k, row in enumerate(scalar_rows):
        def mk(k=k, row=row):
            return nc.tensor.transpose(PBzt[0:64, k:k + 1], row,
                                       ident[0:1, 0:1])
        if k == 0:
            eop("pe", mk, waits=[("dve", E_scp), ("act", slot_last[0])],
                inc=False)
        elif k == len(scalar_rows) - 1:
            E_sctr = eop("pe", mk)
        else:
            mk()
    E_sct = eop("act", lambda: nc.scalar.copy(SCT[0:64, 0:6],
                                              PBzt[0:64, 0:6]),
                waits=[("pe", E_sctr)])

    # fv = stp*(beta - lam1) + (2*lam2*stp)*w_prev   (f16 out)
    e = eop("dve", lambda: TS(fvw[:], wprow[:], DV, None, ALU.mult),
            waits=[("act", E_sct), ("dma_bw", E_bw)])
    e = eop("dve", lambda: TS(fv[:], brow[:], lam1, None, ALU.subtract),
            waits=[("dve", e)])
    E_fv = eop("dve", lambda: STT(fv[:], fv[:], STP, fvw[:], ALU.mult,
                                  ALU.add), waits=[("dve", e)])
    # ws0 = th * w0
    E_ws = eop("dve", lambda: TS(ws[:], wA[:], TH, None, ALU.mult),
               waits=[("dve", max(E_fv, E_wAi))])

    # ---------------------------------------------------------- FISTA
    E_zTcopy = 0
    E_v = 0                       # PByr WAR gate for ytr
    E_z = E_zri
    E_u = 0
    E_v = 0

    for t in range(T):
        wold = wA if t % 2 == 0 else wB
        wnew = wB if t % 2 == 0 else wA
        last = t == T - 1

        if t > 0:
            # ztr: z row (f16) -> PBzt16 col
            gate = (("act", E_sct) if t == 1 else ("dve", E_zTcopy))
            eop("pe", lambda: nc.tensor.transpose(
                PBzt[:, 0:64], zrow[:, 0:128], ident[0:64, 0:64]),
                waits=[("dve", E_z), gate], inc=False)
            E_ztr = eop("pe", lambda: nc.tensor.transpose(
                PBzt[:, 64:128], zrow[:, 128:256], ident[0:64, 0:64]))
            E_zTcopy = eop("dve", lambda: nc.vector.tensor_copy(
                zT[:], PBzt[:, 0:128]), waits=[("pe", E_ztr)])
            E_mm_h0, E_mm = matvec([("dve", E_zTcopy)], tick_each_hi=True)
        else:
            E_mm_h0, E_mm = E_mm0_h0, E_mm0
        # u = ev*z + fv on DVE, scheduled here so it runs under the PE
        # matvec (reads z from the previous iteration's tail)
        E_u = eop("dve", lambda: STT(u[:], zrow[:], EV, fv[:],
                                     ALU.mult, ALU.add),
                  waits=[("dve", max(E_z, E_fv, E_v))])

        # per-half: copy PBy/PBy1 -> ycol (DVE), transpose to PByr (PE):
        # half hi lands at PByr[0:64, hi*128:(hi+1)*128] (row layout).
        E_yc0 = eop("dve", lambda: nc.vector.tensor_copy(
            ycol[:, 0:64], PBy[:, 0:64]), waits=[("pe", E_mm_h0)])
        E_ytr0 = eop("pe", lambda: nc.tensor.transpose(
            PByr[0:64, 0:128], ycol[:, 0:64], ident[:]),
            waits=[("dve", max(E_yc0, E_v))])
        E_yc1 = eop("dve", lambda: nc.vector.tensor_copy(
            ycol[:, 64:128], PBy1[:, 0:64]), waits=[("pe", E_mm)])
        E_ytr = eop("pe", lambda: nc.tensor.transpose(
            PByr[0:64, 128:256], ycol[:, 64:128], ident[:]),
            waits=[("dve", E_yc1)])

        # v = m2a*y + u   (f16 out)
        E_v = eop("dve", lambda: STT(vrow[:], PByr[0:64, 0:256], M2A, u[:],
                                     ALU.mult, ALU.add),
                  waits=[("pe", E_ytr), ("dve", E_u)])

        # Newton: phi = sum max(v,tau) - P*tau - 1; cnt = #(v>tau) + 1
        nsteps = N0 if t == 0 else 1
        for k in range(nsteps):
            e1 = eop("dve", lambda: TS(dumA[:], vrow[:], tau[:], None,
                                       ALU.max, ALU.add, accum_out=S1a[:]),
                     waits=[("dve", max(E_v, E_tau))])
            e2 = eop("dve", lambda: TS(dumB[:], vrow[:], tau[:], None,
                                       ALU.is_gt, ALU.add,
                                       accum_out=cnta[:]),
                     waits=[("dve", e1)])
            e = eop("dve", lambda: TS(X64[:], tau[:], -float(P), -1.0,
                                      ALU.mult, ALU.add),
                    waits=[("dve", e2)])
            e = eop("dve", lambda: TT(G64[:], X64[:], S1a[:], ALU.add),
                    waits=[("dve", e)])
            e3 = eop("dve", lambda: TS(cnta[:], cnta[:], 1.0, None,
                                       ALU.add), waits=[("dve", e)])
            e = eop("dve", lambda: nc.vector.reciprocal(rc[:], cnta[:]),
                    waits=[("dve", e3)])
            E_tau = eop("dve", lambda: STT(tau[:], G64[:], rc[:], tau[:],
                                           ALU.mult, ALU.add),
                        waits=[("dve", e)])

        # w = clip(v - tau, 0, MAX_W)
        e = eop("dve", lambda: TS(t1[:], vrow[:], tau[:], 0.0,
                                  ALU.subtract, ALU.max),
                waits=[("dve", E_tau)])
        E_wn = eop("dve", lambda: TS(wnew[:], t1[:], MAX_W, None, ALU.min),
                   waits=[("dve", e)])

        if not last:
            # z = (1+th)*wnew - ws   (1 op on path); ws = th*wnew off-path
            E_z = eop("dve", lambda: STT(zrow[:], wnew[:], TH1, ws[:],
                                         ALU.mult, ALU.subtract),
                      waits=[("dve", max(E_wn, E_ws))])
            E_ws = eop("dve", lambda: TS(ws[:], wnew[:], TH, None,
                                        ALU.mult), waits=[("dve", E_z)])
        else:
            # renormalize: out = w / (sum(w) + EPS)
            e = eop("dve", lambda: TS(dumA[:], wnew[:], 0.0, None,
                                      ALU.add, ALU.add,
                                      accum_out=ssum[:]),
                    waits=[("dve", E_wn)])
            e = eop("dve", lambda: TS(ssum[:], ssum[:], EPS, None,
                                      ALU.add), waits=[("dve", e)])
            e = eop("dve", lambda: nc.vector.reciprocal(rs[:], ssum[:]),
                    waits=[("dve", e)])
            E_out = eop("dve", lambda: TS(outt[:], wnew[:], rs[:], None,
                                          ALU.mult), waits=[("dve", e)])

    # ---------------------------------------------------------- store
    nc.sync.wait_ge(sems["dve"], E_out)
    d = nc.sync.dma_start(out=out_d, in_=outt[:])
    d.then_inc(sems["dma_out"], 16)
    nc.sync.wait_ge(sems["dma_out"], 16)


def build(lam1, lam2, T=None, npow=None):
    nc = bass.Bass("TRN2", target_bir_lowering=False, debug=False)
    sigma_d = nc.dram_tensor("sigma", [NB, P, P], F32, kind="ExternalInput")
    beta_d = nc.dram_tensor("beta", [NB, P], F32, kind="ExternalInput")
    wprev_d = nc.dram_tensor("w_prev", [NB, P], F32, kind="ExternalInput")
    out_d = nc.dram_tensor("out", [NB, P], F32, kind="ExternalOutput")
    with ExitStack() as ctx:
        _emit(ctx, nc, sigma_d.ap(), beta_d.ap(), wprev_d.ap(), out_d.ap(),
              lam1, lam2, T=T, npow=npow)
    return nc


def kernel(sigma, beta, w_prev, log_lambda1, log_lambda2):
    global LAST_RESULT
    sigma = np.ascontiguousarray(np.asarray(sigma, dtype=np.float32))
    beta = np.ascontiguousarray(np.asarray(beta, dtype=np.float32))
    w_prev = np.ascontiguousarray(np.asarray(w_prev, dtype=np.float32))
    lam1 = float(np.exp(np.float32(log_lambda1)))
    lam2 = float(np.exp(np.float32(log_lambda2)))

    nc = build(lam1, lam2)
    in_maps = []
    for c in range(N_CORES):
        s = slice(c * NB, (c + 1) * NB)
        in_maps.append({
            "sigma": sigma[s],
            "beta": beta[s],
            "w_prev": w_prev[s],
        })
    res = run_bass_kernel_spmd(nc, in_maps, list(range(N_CORES)), trace=TRACE)
    LAST_RESULT = res
    out = np.concatenate([res.results[c]["out"] for c in range(N_CORES)],
                         axis=0)
    return np.ascontiguousarray(out.astype(np.float32))


# revision 39
# speedup vs baseline: 11.3171x; 1.1842x over previous
"""Trainium2 Bass kernel for nn_DifferentiableRiskBudgeting.

Per batch sample b (data parallel, 64 samples per core on 8 cores):
    min_w  w' S_b w - beta_b' w + lam1*||w||_1 + lam2*||w - w_prev||^2
    s.t.   sum w = 1, 0 <= w <= MAX_W
then clamp + renormalize (matching the reference's converged 250-step
PGD fixed point; the QP is strongly convex so the fixed point is
unique).

Algorithm (numpy-validated, rel err ~2.8e-3 vs reference incl. fp16
iterate storage, gate 2e-2):
  - NPOW unnormalized power rounds z <- S z, then Rayleigh quotient
    R = (z3.z2)/(z2.z2) and per-sample step 1/(2*SAFETY*R + 2*lam2).
    FISTA tolerates steps up to 2/L so SAFETY=1.15 on the power
    underestimate is safe (validated worst-case underestimate 1.6x).
  - FISTA with per-sample momentum theta=(1-sqrt(q))/(1+sqrt(q)),
    q = 2*lam2*step, and the scaled-momentum state ws = theta*w so the
    critical-path z update is a single scalar_tensor_tensor:
        z = (1+theta)*w_new - ws;   ws = theta*w_new (off-path)
  - Simplex projection via ONE warm damped-Newton step per iteration
    (N0 cold steps on the first), ignoring the w<=MAX_W cap in the tau
    solve (cap binds for ~3 of 512 samples; the clip + final
    renormalize absorb it — validated):
        phi  = sum max(v,tau) - P*tau - 1
        cnt  = #(v > tau) + 1          (damped slope)
        tau += phi / cnt

Layouts: row [64, 256] (partition = sample, free = feature) for all
elementwise work, so per-sample scalars are per-partition APs and
tensor_scalar accumulations give full per-sample sums directly (walrus
requires matching partition ranges on elementwise operands and
PSUM access patterns starting at partition 0 — no cross-partition
folds are legal).  Column [128, 128] (partition = j within half hj,
free = hj*64 + b) for the PE matvec; sigma symmetric so row-major
chunks double as the transposed stationary operand.

Sigma DMA exploits symmetry: only rows 0:128 (A|B) and the C block are
loaded (75% of bytes); the B^T chunk is reconstructed on-device by PE
transposes + ACT/DVE copies hidden under the DMA (~38us at the modeled
360 GB/s).  Power rounds also pipeline per 8-sample batch under the
DMA, so the spectral estimate is free.

Iterate tensors are fp16 (DVE 4x mode on the clip/reduction passes);
the matvec path (sigma, zT, PSUM) stays fp32.  The y halves land in
two separate PSUM banks so a consumer read of half 0 never shares a
bank with the in-flight PE writes of half 1 (same-bank PE-write +
engine-read is fatal on HW).

Raw bass with explicit semaphores; waits are FUSED onto the consuming
instruction (1 wait + 1 inc <= walrus' ~2 sync commands per
instruction), extra waits standalone.  Same-engine dependent ops also
need inc+wait pairs (engine pipelines do not interlock); ordering is
transitive through any later same-engine inc.
"""

import math
import numpy as np
from contextlib import ExitStack

import concourse.bass as bass
from concourse import mybir
from concourse.bass_utils import run_bass_kernel_spmd

F32 = mybir.dt.float32
F16 = mybir.dt.float16
ALU = mybir.AluOpType
ACTF = mybir.ActivationFunctionType

B, P = 512, 256
N_CORES = 8
NB = B // N_CORES            # samples per core (64)
H = 2                        # feature halves
MAX_W = 0.1
EPS = 1e-8

NPOW = 2                     # unnormalized power rounds
SAFETY = 1.3
T_FISTA = 18
N0 = 2                       # Newton steps, first iteration
SIG_DMA_BATCH = 4            # samples per sigma DMA
POW_BATCH = 8                # samples per pipelined power batch

TRACE = False
LAST_RESULT = None


def _emit(ctx, nc, sigma_d, beta_d, wprev_d, out_d, lam1, lam2,
          T=None, npow=None):
    nb = NB
    T = T_FISTA if T is None else T
    npow_ = NPOW if npow is None else npow

    def sbuf(name, shape, dt=F32):
        return ctx.enter_context(nc.sbuf_tensor(name, shape, dt))

    def psum(name):
        return ctx.enter_context(nc.psum_tensor(name, [128, 512], F32))

    ENG = {"pe": nc.tensor, "dve": nc.vector, "act": nc.scalar,
           "pool": nc.gpsimd, "sync": nc.sync}
    sems = {e: ctx.enter_context(nc.semaphore(f"s_{e}"))
            for e in ("pe", "dve", "act", "pool", "dma_bw", "dma_out")}
    npb = NB // POW_BATCH
    for g in range(npb):
        sems[f"dma_sig{g}"] = ctx.enter_context(nc.semaphore(f"s_dsig{g}"))
    ctr = {e: 0 for e in sems}
    last_wait = {e: {} for e in list(ENG)}

    def eop(eng, emit, waits=(), inc=True):
        """Emit an op on `eng` with fused sync (see module docstring)."""
        lw = last_wait[eng]
        pend = []
        for s, v in waits:
            if v and v > 0 and lw.get(s, 0) < v:
                pend.append((s, v))
                lw[s] = v
        for s, v in pend[1:]:
            ENG[eng].wait_ge(sems[s], v)
        inst = emit()
        if pend:
            s, v = pend[0]
            inst.wait_op(sems[s], v, "sem-ge")
        if inc:
            ctr[eng] += 1
            inst.then_inc(sems[eng], 1)
            return ctr[eng]
        return None

    # ------------------------------------------------------------- tensors
    nbatch = SIG_DMA_BATCH
    nk = nb // nbatch
    sig = [sbuf(f"sig{k}", [128, nbatch * H * P]) for k in range(nk)]

    def sig_ap(b, hj, hi):
        k, m = divmod(b, nbatch)
        c0 = (m * H + hj) * P + hi * 128
        return sig[k][:, c0:c0 + 128]

    ident = sbuf("ident", [128, 128])
    ones = sbuf("ones", [128, 1])
    zT = sbuf("zT", [128, 128])          # col: [j-in-half, hj*64+b], f32
    ycol = sbuf("ycol", [128, 128])      # col: [i-in-half, hi*64+b], f32
    pq = sbuf("pq", [128, 128])
    qq = sbuf("qq", [128, 128])
    zrow = sbuf("zrow", [64, 256])       # row: [b, j]
    u = sbuf("u", [64, 256])
    fv = sbuf("fv", [64, 256])
    fvw = sbuf("fvw", [64, 256])
    vrow = sbuf("vrow", [64, 256], F16)
    t1 = sbuf("t1", [64, 256], F16)
    wA = sbuf("wA", [64, 256], F16)
    wB = sbuf("wB", [64, 256], F16)
    ws = sbuf("ws", [64, 256], F16)
    brow = sbuf("brow", [64, 256])
    wprow = sbuf("wprow", [64, 256])
    outt = sbuf("outt", [64, 256])
    dumA = sbuf("dumA", [64, 256], F16)
    dumB = sbuf("dumB", [64, 256], F16)
    SCT = sbuf("SCT", [64, 8])           # cols: stp, ev, m2a, dv, th, th1
    SCX = sbuf("SCX", [1, 1024])         # [1,64] scratch slices
    tau = sbuf("tau", [64, 1])
    S1a = sbuf("S1a", [64, 1])
    cnta = sbuf("cnta", [64, 1])
    X64 = sbuf("X64", [64, 1])
    G64 = sbuf("G64", [64, 1])
    rc = sbuf("rc", [64, 1])
    ssum = sbuf("ssum", [64, 1])
    rs = sbuf("rs", [64, 1])

    PBzt = psum("PBzt")
    PBy = psum("PBy")
    PBy1 = psum("PBy1")      # hi=1 matvec bank (bank-isolated from PBy)
    PByr = psum("PByr")
    PBr = psum("PBr")

    STP = SCT[:, 0:1]
    EV = SCT[:, 1:2]
    M2A = SCT[:, 2:3]
    DV = SCT[:, 3:4]
    TH = SCT[:, 4:5]
    TH1 = SCT[:, 5:6]

    def scx(k):
        return SCX[0:1, 64 * k:64 * (k + 1)]

    # --------------------------------------------------------------- DMAs
    # sigma symmetric: load A|B rows + C block; B^T rebuilt on-device.
    kb_per_g = nk // npb
    for k in range(nk):
        ks = sigma_d[k * nbatch:(k + 1) * nbatch]
        dst4 = sig[k][:].rearrange("p (b h j) -> p b h j", b=nbatch, h=H)
        d = nc.sync.dma_start(
            out=dst4[:, :, 0, :],
            in_=ks[:, 0:128, :].rearrange("b p j -> p b j"))
        d.then_inc(sems[f"dma_sig{k // kb_per_g}"], 16)
        d = nc.sync.dma_start(
            out=dst4[:, :, 1, 128:256],
            in_=ks[:, 128:256, 128:256].rearrange("b p j -> p b j"))
        d.then_inc(sems[f"dma_sig{k // kb_per_g}"], 16)
    E_sig_g = 32 * kb_per_g
    d = nc.sync.dma_start(out=brow[:], in_=beta_d)
    d.then_inc(sems["dma_bw"], 16)
    d = nc.sync.dma_start(out=wprow[:], in_=wprev_d)
    d.then_inc(sems["dma_bw"], 16)
    E_bw = 32

    # ------------------------------------------------------------ preamble
    eop("dve", lambda: nc.vector.memset(ident[:], 0.0))
    eop("dve", lambda: nc.vector.memset(ones[:], 1.0))
    E_zT0 = eop("dve", lambda: nc.vector.memset(zT[:], 1.0 / math.sqrt(P)))
    E_ident = eop("pool", lambda: nc.gpsimd.affine_select(
        out=ident[:], in_=ident[:], compare_op=ALU.not_equal, fill=1.0,
        base=0, pattern=[[-1, 128]], channel_multiplier=1),
        waits=[("dve", E_zT0)])

    def matvec(gate_waits, b_range=range(nb), tick_each_hi=False):
        """Column-space matvec: y-half hi of sample b -> (PBy|PBy1)[:, b].

        Returns (tick_hi0, tick_hi1_mid, tick_all) pe ticks."""
        t_hi0 = t_mid = None
        bmid = b_range[len(b_range) // 2 - 1]
        first = True
        for hi in range(H):
            dst = PBy if hi == 0 else PBy1
            for b in b_range:
                for hj in range(H):
                    def mk(b=b, hj=hj, hi=hi, dst=dst):
                        return nc.tensor.matmul(
                            dst[:, b:b + 1], sig_ap(b, hj, hi),
                            zT[:, hj * 64 + b:hj * 64 + b + 1],
                            start=(hj == 0), stop=(hj == H - 1))
                    is_last = (b == b_range[-1] and hj == H - 1)
                    is_mid = (hi == H - 1 and b == bmid and hj == H - 1)
                    if first:
                        tick = eop("pe", mk, waits=gate_waits)
                        first = False
                    elif is_last and (hi == H - 1 or tick_each_hi):
                        tick = eop("pe", mk)
                    elif is_mid and tick_each_hi:
                        t_mid = eop("pe", mk)
                    else:
                        mk()
            if hi == 0:
                t_hi0 = tick if tick_each_hi else None
        return t_hi0, t_mid, tick

    # ------------------------------- B^T reconstruction + power (pipelined)
    E_pmm_last = 0
    slot_last = {0: 0, 1: 0}          # last B-copy tick per slot (act/dve)
    for pb in range(npb):
        b0 = pb * POW_BATCH
        for bb in range(POW_BATCH):
            b = b0 + bb
            s = bb % 2
            bank = PBzt if s == 0 else PBr
            ceng = "act" if s == 0 else "dve"

            def tr(b=b, bank=bank):
                return nc.tensor.transpose(bank[:, 0:128],
                                           sig_ap(b, 0, 1), ident[:])
            E_tr = eop("pe", tr, waits=[
                (f"dma_sig{pb}", E_sig_g), ("pool", E_ident),
                (ceng, slot_last[s])])

            def cp(b=b, bank=bank, ceng=ceng):
                op = (nc.scalar.copy if ceng == "act"
                      else nc.vector.tensor_copy)
                return op(sig_ap(b, 1, 0), bank[:, 0:128])
            slot_last[s] = eop(ceng, cp, waits=[("pe", E_tr)])
        E_bt_act, E_bt_dve = slot_last[0], slot_last[1]

        E_cp = 0
        for r in range(npow_):
            w = [("pool", E_ident), ("dve", max(E_zT0, E_bt_dve)),
                 ("act", max(E_cp, E_bt_act))]
            _, _, E_mm = matvec(w, b_range=range(b0, b0 + POW_BATCH))
            if r < npow_ - 1:
                def cp0(b0=b0):
                    return nc.scalar.copy(
                        zT[:].rearrange("p (h b) -> p h b", h=H)[
                            :, 0, b0:b0 + POW_BATCH],
                        PBy[:, b0:b0 + POW_BATCH])

                def cp1(b0=b0):
                    return nc.scalar.copy(
                        zT[:].rearrange("p (h b) -> p h b", h=H)[
                            :, 1, b0:b0 + POW_BATCH],
                        PBy1[:, b0:b0 + POW_BATCH])
                eop("act", cp0, waits=[("pe", E_mm)])
                E_cp = eop("act", cp1)
            else:
                E_pmm_last = E_mm

    # --------------------------------------------------------- Rayleigh
    eop("dve", lambda: nc.vector.tensor_tensor(
        pq[:, 0:64], PBy[:, 0:64], zT[:, 0:64], ALU.mult),
        waits=[("pe", E_pmm_last)])
    E_pq = eop("dve", lambda: nc.vector.tensor_tensor(
        pq[:, 64:128], PBy1[:, 0:64], zT[:, 64:128], ALU.mult))
    E_qq = eop("dve", lambda: nc.vector.tensor_tensor(
        qq[:], zT[:], zT[:], ALU.mult), waits=[("dve", E_pq)])
    eop("pe", lambda: nc.tensor.matmul(
        PBr[0:1, 0:128], ones[:], pq[:], start=True, stop=True),
        waits=[("dve", E_qq)])
    E_red = eop("pe", lambda: nc.tensor.matmul(
        PBr[0:1, 128:256], ones[:], qq[:], start=True, stop=True))

    # t=0 state init (zT reused: re-memset after qq consumed it)
    E_zTi = eop("dve", lambda: nc.vector.memset(zT[:], 1.0 / P),
                waits=[("dve", E_qq)])
    E_zri = eop("dve", lambda: nc.vector.memset(zrow[:], 1.0 / P))
    E_wAi = eop("dve", lambda: nc.vector.memset(wA[:], 1.0 / P))
    eop("dve", lambda: nc.vector.memset(tau[:], 0.0))
    E_tau = eop("dve", lambda: nc.vector.memset(X64[:], -1.0))

    # t=0 matvec (emitted early; PE works while DVE does the scalar chain)
    E_mm0_h0, E_mm0_mid, E_mm0 = matvec([("dve", E_zTi)], tick_each_hi=True)

    # ------------------------------------------------------ scalar chain
    TS = nc.vector.tensor_scalar
    TT = nc.vector.tensor_tensor
    STT = nc.vector.scalar_tensor_tensor
    num, den, rden, R, L = scx(0), scx(1), scx(2), scx(3), scx(4)
    sq, onep, rop, onem = scx(5), scx(6), scx(7), scx(8)
    stp_r, ev_r, m2a_r, dv_r, th_r, th1_r = (scx(9), scx(10), scx(11),
                                             scx(12), scx(13), scx(14))

    prs = SCX[0:1, 768:1024]   # SBUF copy of the PE-reduce results
    e = eop("dve", lambda: nc.vector.tensor_copy(prs, PBr[0:1, 0:256]),
            waits=[("pe", E_red)])
    e = eop("dve", lambda: TT(num, SCX[0:1, 768:832], SCX[0:1, 832:896],
                              ALU.add), waits=[("dve", e)])
    e = eop("dve", lambda: TT(den, SCX[0:1, 896:960], SCX[0:1, 960:1024],
                              ALU.add), waits=[("dve", e)])
    e = eop("dve", lambda: TS(den, den, EPS, None, ALU.add),
            waits=[("dve", e)])
    e = eop("dve", lambda: nc.vector.reciprocal(rden, den),
            waits=[("dve", e)])
    e = eop("dve", lambda: TT(R, num, rden, ALU.mult), waits=[("dve", e)])
    e = eop("dve", lambda: TS(L, R, 2.0 * SAFETY, 2.0 * lam2 + 1e-6,
                              ALU.mult, ALU.add), waits=[("dve", e)])
    e = eop("dve", lambda: nc.vector.reciprocal(stp_r, L),
            waits=[("dve", e)])
    e = eop("dve", lambda: TS(ev_r, stp_r, -2.0 * lam2, 1.0, ALU.mult,
                              ALU.add), waits=[("dve", e)])
    e = eop("dve", lambda: TS(m2a_r, stp_r, -2.0, None, ALU.mult),
            waits=[("dve", e)])
    E_dv = eop("dve", lambda: TS(dv_r, stp_r, 2.0 * lam2, None, ALU.mult),
               waits=[("dve", e)])
    E_sq = eop("act", lambda: nc.scalar.activation(sq, dv_r, ACTF.Sqrt),
               waits=[("dve", E_dv)])
    e = eop("dve", lambda: TS(onep, sq, 1.0, None, ALU.add),
            waits=[("act", E_sq)])
    e = eop("dve", lambda: nc.vector.reciprocal(rop, onep),
            waits=[("dve", e)])
    e = eop("dve", lambda: TS(onem, sq, -1.0, 1.0, ALU.mult, ALU.add),
            waits=[("dve", e)])
    e = eop("dve", lambda: TT(th_r, onem, rop, ALU.mult),
            waits=[("dve", e)])
    E_scp = eop("dve", lambda: TS(th1_r, th_r, 1.0, None, ALU.add),
                waits=[("dve", e)])
    # transpose each [1,64] scalar row -> [64,1]; land in PBzt cols 0..5
    scalar_rows = [stp_r, ev_r, m2a_r, dv_r, th_r, th1_r]
    for k, row in enumerate(scalar_rows):
        def mk(k=k, row=row):
            return nc.tensor.transpose(PBzt[0:64, k:k + 1], row,
                                       ident[0:1, 0:1])
        if k == 0:
            eop("pe", mk, waits=[("dve", E_scp), ("act", slot_last[0])],
                inc=False)
        elif k == len(scalar_rows) - 1:
            E_sctr = eop("pe", mk)
        else:
            mk()
    E_sct = eop("act", lambda: nc.scalar.copy(SCT[0:64, 0:6],
                                              PBzt[0:64, 0:6]),
                waits=[("pe", E_sctr)])

    # fv = stp*(beta - lam1) + (2*lam2*stp)*w_prev   (f16 out)
    e = eop("dve", lambda: TS(fvw[:], wprow[:], DV, None, ALU.mult),
            waits=[("act", E_sct), ("dma_bw", E_bw)])
    e = eop("dve", lambda: TS(fv[:], brow[:], lam1, None, ALU.subtract),
            waits=[("dve", e)])
    E_fv = eop("dve", lambda: STT(fv[:], fv[:], STP, fvw[:], ALU.mult,
                                  ALU.add), waits=[("dve", e)])
    # ws0 = th * w0
    E_ws = eop("dve", lambda: TS(ws[:], wA[:], TH, None, ALU.mult),
               waits=[("dve", max(E_fv, E_wAi))])

    # ---------------------------------------------------------- FISTA
    E_zTcopy = 0
    E_v = 0                       # PByr WAR gate for ytr
    E_z = E_zri
    E_u = 0
    E_v = 0

    for t in range(T):
        wold = wA if t % 2 == 0 else wB
        wnew = wB if t % 2 == 0 else wA
        last = t == T - 1

        if t > 0:
            # ztr: z row (f16) -> PBzt16 col
            gate = (("act", E_sct) if t == 1 else ("dve", E_zTcopy))
            eop("pe", lambda: nc.tensor.transpose(
                PBzt[:, 0:64], zrow[:, 0:128], ident[0:64, 0:64]),
                waits=[("dve", E_z), gate], inc=False)
            E_ztr = eop("pe", lambda: nc.tensor.transpose(
                PBzt[:, 64:128], zrow[:, 128:256], ident[0:64, 0:64]))
            E_zTcopy = eop("dve", lambda: nc.vector.tensor_copy(
                zT[:], PBzt[:, 0:128]), waits=[("pe", E_ztr)])
            E_mm_h0, E_mm_mid, E_mm = matvec([("dve", E_zTcopy)],
                                             tick_each_hi=True)
        else:
            E_mm_h0, E_mm_mid, E_mm = E_mm0_h0, E_mm0_mid, E_mm0
        # u = ev*z + fv on DVE, scheduled here so it runs under the PE
        # matvec (reads z from the previous iteration's tail)
        E_u = eop("dve", lambda: STT(u[:], zrow[:], EV, fv[:],
                                     ALU.mult, ALU.add),
                  waits=[("dve", max(E_z, E_fv, E_v))])

        # per-half: copy PBy/PBy1 -> ycol (DVE), transpose to PByr (PE):
        # half hi lands at PByr[0:64, hi*128:(hi+1)*128] (row layout).
        E_yc0 = eop("dve", lambda: nc.vector.tensor_copy(
            ycol[:, 0:64], PBy[:, 0:64]), waits=[("pe", E_mm_h0)])
        E_ytr0 = eop("pe", lambda: nc.tensor.transpose(
            PByr[0:64, 0:128], ycol[:, 0:64], ident[:]),
            waits=[("dve", max(E_yc0, E_v))])
        eop("dve", lambda: nc.vector.tensor_copy(
            ycol[:, 64:96], PBy1[:, 0:32]), waits=[("pe", E_mm_mid)])
        E_yc1 = eop("dve", lambda: nc.vector.tensor_copy(
            ycol[:, 96:128], PBy1[:, 32:64]), waits=[("pe", E_mm)])
        E_ytr = eop("pe", lambda: nc.tensor.transpose(
            PByr[0:64, 128:256], ycol[:, 64:128], ident[:]),
            waits=[("dve", E_yc1)])

        # v = m2a*y + u   (f16 out)
        E_v = eop("dve", lambda: STT(vrow[:], PByr[0:64, 0:256], M2A, u[:],
                                     ALU.mult, ALU.add),
                  waits=[("pe", E_ytr), ("dve", E_u)])

        # Newton: phi = sum max(v,tau) - P*tau - 1; cnt = #(v>tau) + 1
        # X64 = -P*tau - 1 is precomputed off the critical path (end of
        # the previous iteration / inner step), so the v->tau chain is
        # S1,cnt,G,rc,tau.
        nsteps = N0 if t == 0 else 1
        if t == 0:
            # analytic warm start: tau0 = (sum(v) - 1)/P
            e0 = eop("dve", lambda: TS(dumA[:], vrow[:], 0.0, None,
                                       ALU.add, ALU.add,
                                       accum_out=S1a[:]),
                     waits=[("dve", max(E_v, E_tau))])
            e0 = eop("dve", lambda: TS(tau[:], S1a[:], 1.0 / P,
                                       -1.0 / P, ALU.mult, ALU.add),
                     waits=[("dve", e0)])
            E_tau = eop("dve", lambda: TS(X64[:], tau[:], -float(P),
                                          -1.0, ALU.mult, ALU.add),
                        waits=[("dve", e0)])
        for k in range(nsteps):
            e1 = eop("dve", lambda: TS(dumA[:], vrow[:], tau[:], None,
                                       ALU.max, ALU.add, accum_out=S1a[:]),
                     waits=[("dve", max(E_v, E_tau))])
            e2 = eop("dve", lambda: TS(dumB[:], vrow[:], tau[:], None,
                                       ALU.is_gt, ALU.add,
                                       accum_out=cnta[:]),
                     waits=[("dve", e1)])
            e = eop("dve", lambda: TT(G64[:], X64[:], S1a[:], ALU.add),
                    waits=[("dve", e2)])
            e = eop("dve", lambda: nc.vector.reciprocal(rc[:], cnta[:]),
                    waits=[("dve", e)])
            E_tau = eop("dve", lambda: STT(tau[:], G64[:], rc[:], tau[:],
                                           ALU.mult, ALU.add),
                        waits=[("dve", e)])
            if k < nsteps - 1:
                E_tau = eop("dve", lambda: TS(X64[:], tau[:], -float(P),
                                              -1.0, ALU.mult, ALU.add),
                            waits=[("dve", E_tau)])

        # w = clip(v - tau, 0, MAX_W)
        e = eop("dve", lambda: TS(t1[:], vrow[:], tau[:], 0.0,
                                  ALU.subtract, ALU.max),
                waits=[("dve", E_tau)])
        E_wn = eop("dve", lambda: TS(wnew[:], t1[:], MAX_W, None, ALU.min),
                   waits=[("dve", e)])

        if not last:
            # z = (1+th)*wnew - ws   (1 op on path); ws = th*wnew off-path
            E_z = eop("dve", lambda: STT(zrow[:], wnew[:], TH1, ws[:],
                                         ALU.mult, ALU.subtract),
                      waits=[("dve", max(E_wn, E_ws))])
            E_ws = eop("dve", lambda: TS(ws[:], wnew[:], TH, None,
                                        ALU.mult), waits=[("dve", E_z)])
            eop("dve", lambda: TS(X64[:], tau[:], -float(P), -1.0,
                                  ALU.mult, ALU.add),
                waits=[("dve", E_ws)])
        else:
            # renormalize: out = w / (sum(w) + EPS)
            e = eop("dve", lambda: TS(dumA[:], wnew[:], 0.0, None,
                                      ALU.add, ALU.add,
                                      accum_out=ssum[:]),
                    waits=[("dve", E_wn)])
            e = eop("dve", lambda: TS(ssum[:], ssum[:], EPS, None,
                                      ALU.add), waits=[("dve", e)])
            e = eop("dve", lambda: nc.vector.reciprocal(rs[:], ssum[:]),
                    waits=[("dve", e)])
            E_out = eop("dve", lambda: TS(outt[:], wnew[:], rs[:], None,
                                          ALU.mult), waits=[("dve", e)])

    # ---------------------------------------------------------- store
    nc.sync.wait_ge(sems["dve"], E_out)
    d = nc.sync.dma_start(out=out_d, in_=outt[:])
    d.then_inc(sems["dma_out"], 16)
    nc.sync.wait_ge(sems["dma_out"], 16)


def build(lam1, lam2, T=None, npow=None):
    nc = bass.Bass("TRN2", target_bir_lowering=False, debug=False)
    sigma_d = nc.dram_tensor("sigma", [NB, P, P], F32, kind="ExternalInput")
    beta_d = nc.dram_tensor("beta", [NB, P], F32, kind="ExternalInput")
    wprev_d = nc.dram_tensor("w_prev", [NB, P], F32, kind="ExternalInput")
    out_d = nc.dram_tensor("out", [NB, P], F32, kind="ExternalOutput")
    with ExitStack() as ctx:
        _emit(ctx, nc, sigma_d.ap(), beta_d.ap(), wprev_d.ap(), out_d.ap(),
              lam1, lam2, T=T, npow=npow)
    return nc


def kernel(sigma, beta, w_prev, log_lambda1, log_lambda2):
    global LAST_RESULT
    sigma = np.ascontiguousarray(np.asarray(sigma, dtype=np.float32))
    beta = np.ascontiguousarray(np.asarray(beta, dtype=np.float32))
    w_prev = np.ascontiguousarray(np.asarray(w_prev, dtype=np.float32))
    lam1 = float(np.exp(np.float32(log_lambda1)))
    lam2 = float(np.exp(np.float32(log_lambda2)))

    nc = build(lam1, lam2)
    in_maps = []
    for c in range(N_CORES):
        s = slice(c * NB, (c + 1) * NB)
        in_maps.append({
            "sigma": sigma[s],
            "beta": beta[s],
            "w_prev": w_prev[s],
        })
    res = run_bass_kernel_spmd(nc, in_maps, list(range(N_CORES)), trace=TRACE)
    LAST_RESULT = res
    out = np.concatenate([res.results[c]["out"] for c in range(N_CORES)],
                         axis=0)
    return np.ascontiguousarray(out.astype(np.float32))


# revision 40
# speedup vs baseline: 11.5382x; 1.0195x over previous
"""Trainium2 Bass kernel for nn_DifferentiableRiskBudgeting.

Per batch sample b (data parallel, 64 samples per core on 8 cores):
    min_w  w' S_b w - beta_b' w + lam1*||w||_1 + lam2*||w - w_prev||^2
    s.t.   sum w = 1, 0 <= w <= MAX_W
then clamp + renormalize (matching the reference's converged 250-step
PGD fixed point; the QP is strongly convex so the fixed point is
unique).

Algorithm (numpy-validated, rel err ~2.8e-3 vs reference incl. fp16
iterate storage, gate 2e-2):
  - NPOW unnormalized power rounds z <- S z, then Rayleigh quotient
    R = (z3.z2)/(z2.z2) and per-sample step 1/(2*SAFETY*R + 2*lam2).
    FISTA tolerates steps up to 2/L so SAFETY=1.15 on the power
    underestimate is safe (validated worst-case underestimate 1.6x).
  - FISTA with per-sample momentum theta=(1-sqrt(q))/(1+sqrt(q)),
    q = 2*lam2*step, and the scaled-momentum state ws = theta*w so the
    critical-path z update is a single scalar_tensor_tensor:
        z = (1+theta)*w_new - ws;   ws = theta*w_new (off-path)
  - Simplex projection via ONE warm damped-Newton step per iteration
    (N0 cold steps on the first), ignoring the w<=MAX_W cap in the tau
    solve (cap binds for ~3 of 512 samples; the clip + final
    renormalize absorb it — validated):
        phi  = sum max(v,tau) - P*tau - 1
        cnt  = #(v > tau) + 1          (damped slope)
        tau += phi / cnt

Layouts: row [64, 256] (partition = sample, free = feature) for all
elementwise work, so per-sample scalars are per-partition APs and
tensor_scalar accumulations give full per-sample sums directly (walrus
requires matching partition ranges on elementwise operands and
PSUM access patterns starting at partition 0 — no cross-partition
folds are legal).  Column [128, 128] (partition = j within half hj,
free = hj*64 + b) for the PE matvec; sigma symmetric so row-major
chunks double as the transposed stationary operand.

Sigma DMA exploits symmetry: only rows 0:128 (A|B) and the C block are
loaded (75% of bytes); the B^T chunk is reconstructed on-device by PE
transposes + ACT/DVE copies hidden under the DMA (~38us at the modeled
360 GB/s).  Power rounds also pipeline per 8-sample batch under the
DMA, so the spectral estimate is free.

Iterate tensors are fp16 (DVE 4x mode on the clip/reduction passes);
the matvec path (sigma, zT, PSUM) stays fp32.  The y halves land in
two separate PSUM banks so a consumer read of half 0 never shares a
bank with the in-flight PE writes of half 1 (same-bank PE-write +
engine-read is fatal on HW).

Raw bass with explicit semaphores; waits are FUSED onto the consuming
instruction (1 wait + 1 inc <= walrus' ~2 sync commands per
instruction), extra waits standalone.  Same-engine dependent ops also
need inc+wait pairs (engine pipelines do not interlock); ordering is
transitive through any later same-engine inc.
"""

import math
import numpy as np
from contextlib import ExitStack

import concourse.bass as bass
from concourse import mybir
from concourse.bass_utils import run_bass_kernel_spmd

F32 = mybir.dt.float32
F16 = mybir.dt.float16
ALU = mybir.AluOpType
ACTF = mybir.ActivationFunctionType

B, P = 512, 256
N_CORES = 8
NB = B // N_CORES            # samples per core (64)
H = 2                        # feature halves
MAX_W = 0.1
EPS = 1e-8

NPOW = 2                     # unnormalized power rounds
SAFETY = 1.3
T_FISTA = 18
N0 = 2                       # Newton steps, first iteration
SIG_DMA_BATCH = 4            # samples per sigma DMA
POW_BATCH = 8                # samples per pipelined power batch

TRACE = False
LAST_RESULT = None


def _emit(ctx, nc, sigma_d, beta_d, wprev_d, out_d, lam1, lam2,
          T=None, npow=None):
    nb = NB
    T = T_FISTA if T is None else T
    npow_ = NPOW if npow is None else npow

    def sbuf(name, shape, dt=F32):
        return ctx.enter_context(nc.sbuf_tensor(name, shape, dt))

    def psum(name):
        return ctx.enter_context(nc.psum_tensor(name, [128, 512], F32))

    ENG = {"pe": nc.tensor, "dve": nc.vector, "act": nc.scalar,
           "pool": nc.gpsimd, "sync": nc.sync}
    sems = {e: ctx.enter_context(nc.semaphore(f"s_{e}"))
            for e in ("pe", "dve", "act", "pool", "dma_bw", "dma_out")}
    npb = NB // POW_BATCH
    for g in range(npb):
        sems[f"dma_sig{g}"] = ctx.enter_context(nc.semaphore(f"s_dsig{g}"))
    ctr = {e: 0 for e in sems}
    last_wait = {e: {} for e in list(ENG)}

    def eop(eng, emit, waits=(), inc=True):
        """Emit an op on `eng` with fused sync (see module docstring)."""
        lw = last_wait[eng]
        pend = []
        for s, v in waits:
            if v and v > 0 and lw.get(s, 0) < v:
                pend.append((s, v))
                lw[s] = v
        for s, v in pend[1:]:
            ENG[eng].wait_ge(sems[s], v)
        inst = emit()
        if pend:
            s, v = pend[0]
            inst.wait_op(sems[s], v, "sem-ge")
        if inc:
            ctr[eng] += 1
            inst.then_inc(sems[eng], 1)
            return ctr[eng]
        return None

    # ------------------------------------------------------------- tensors
    nbatch = SIG_DMA_BATCH
    nk = nb // nbatch
    sig = [sbuf(f"sig{k}", [128, nbatch * H * P]) for k in range(nk)]

    def sig_ap(b, hj, hi):
        k, m = divmod(b, nbatch)
        c0 = (m * H + hj) * P + hi * 128
        return sig[k][:, c0:c0 + 128]

    ident = sbuf("ident", [128, 128])
    ones = sbuf("ones", [128, 1])
    zT = sbuf("zT", [128, 128])          # col: [j-in-half, hj*64+b], f32
    ycol = sbuf("ycol", [128, 128])      # col: [i-in-half, hi*64+b], f32
    pq = sbuf("pq", [128, 128])
    qq = sbuf("qq", [128, 128])
    zrow = sbuf("zrow", [64, 256])       # row: [b, j]
    u = sbuf("u", [64, 256])
    fv = sbuf("fv", [64, 256])
    fvw = sbuf("fvw", [64, 256])
    vrow = sbuf("vrow", [64, 256], F16)
    t1 = sbuf("t1", [64, 256], F16)
    wA = sbuf("wA", [64, 256], F16)
    wB = sbuf("wB", [64, 256], F16)
    ws = sbuf("ws", [64, 256], F16)
    brow = sbuf("brow", [64, 256])
    wprow = sbuf("wprow", [64, 256])
    outt = sbuf("outt", [64, 256])
    dumA = sbuf("dumA", [64, 256], F16)
    dumB = sbuf("dumB", [64, 256], F16)
    SCT = sbuf("SCT", [64, 8])           # cols: stp, ev, m2a, dv, th, th1
    SCX = sbuf("SCX", [1, 1024])         # [1,64] scratch slices
    tau = sbuf("tau", [64, 1])
    S1a = sbuf("S1a", [64, 1])
    cnta = sbuf("cnta", [64, 1])
    X64 = sbuf("X64", [64, 1])
    G64 = sbuf("G64", [64, 1])
    rc = sbuf("rc", [64, 1])
    ssum = sbuf("ssum", [64, 1])
    rs = sbuf("rs", [64, 1])

    PBzt = psum("PBzt")
    PBy = psum("PBy")
    PBy1 = psum("PBy1")      # hi=1 matvec bank (bank-isolated from PBy)
    PByr = psum("PByr")
    PBr = psum("PBr")

    STP = SCT[:, 0:1]
    EV = SCT[:, 1:2]
    M2A = SCT[:, 2:3]
    DV = SCT[:, 3:4]
    TH = SCT[:, 4:5]
    TH1 = SCT[:, 5:6]

    def scx(k):
        return SCX[0:1, 64 * k:64 * (k + 1)]

    # --------------------------------------------------------------- DMAs
    # sigma symmetric: load A|B rows + C block; B^T rebuilt on-device.
    kb_per_g = nk // npb
    for k in range(nk):
        ks = sigma_d[k * nbatch:(k + 1) * nbatch]
        dst4 = sig[k][:].rearrange("p (b h j) -> p b h j", b=nbatch, h=H)
        d = nc.sync.dma_start(
            out=dst4[:, :, 0, :],
            in_=ks[:, 0:128, :].rearrange("b p j -> p b j"))
        d.then_inc(sems[f"dma_sig{k // kb_per_g}"], 16)
        d = nc.sync.dma_start(
            out=dst4[:, :, 1, 128:256],
            in_=ks[:, 128:256, 128:256].rearrange("b p j -> p b j"))
        d.then_inc(sems[f"dma_sig{k // kb_per_g}"], 16)
    E_sig_g = 32 * kb_per_g
    d = nc.sync.dma_start(out=brow[:], in_=beta_d)
    d.then_inc(sems["dma_bw"], 16)
    d = nc.sync.dma_start(out=wprow[:], in_=wprev_d)
    d.then_inc(sems["dma_bw"], 16)
    E_bw = 32

    # ------------------------------------------------------------ preamble
    eop("dve", lambda: nc.vector.memset(ident[:], 0.0))
    eop("dve", lambda: nc.vector.memset(ones[:], 1.0))
    E_zT0 = eop("dve", lambda: nc.vector.memset(zT[:], 1.0 / math.sqrt(P)))
    E_ident = eop("pool", lambda: nc.gpsimd.affine_select(
        out=ident[:], in_=ident[:], compare_op=ALU.not_equal, fill=1.0,
        base=0, pattern=[[-1, 128]], channel_multiplier=1),
        waits=[("dve", E_zT0)])

    def matvec(gate_waits, b_range=range(nb), tick_each_hi=False):
        """Column-space matvec: y-half hi of sample b -> (PBy|PBy1)[:, b].

        Returns (tick_hi0, tick_hi1_mid, tick_all) pe ticks."""
        t_hi0 = t_mid = None
        bmid = b_range[len(b_range) // 2 - 1]
        first = True
        for hi in range(H):
            dst = PBy if hi == 0 else PBy1
            for b in b_range:
                for hj in range(H):
                    def mk(b=b, hj=hj, hi=hi, dst=dst):
                        return nc.tensor.matmul(
                            dst[:, b:b + 1], sig_ap(b, hj, hi),
                            zT[:, hj * 64 + b:hj * 64 + b + 1],
                            start=(hj == 0), stop=(hj == H - 1))
                    is_last = (b == b_range[-1] and hj == H - 1)
                    is_mid = (hi == H - 1 and b == bmid and hj == H - 1)
                    if first:
                        tick = eop("pe", mk, waits=gate_waits)
                        first = False
                    elif is_last and (hi == H - 1 or tick_each_hi):
                        tick = eop("pe", mk)
                    elif is_mid and tick_each_hi:
                        t_mid = eop("pe", mk)
                    else:
                        mk()
            if hi == 0:
                t_hi0 = tick if tick_each_hi else None
        return t_hi0, t_mid, tick

    # ------------------------------- B^T reconstruction + power (pipelined)
    E_pmm_last = 0
    slot_last = {0: 0, 1: 0}          # last B-copy tick per slot (act/dve)
    for pb in range(npb):
        b0 = pb * POW_BATCH
        for bb in range(POW_BATCH):
            b = b0 + bb
            s = bb % 2
            bank = PBzt if s == 0 else PBr
            ceng = "act" if s == 0 else "dve"

            def tr(b=b, bank=bank):
                return nc.tensor.transpose(bank[:, 0:128],
                                           sig_ap(b, 0, 1), ident[:])
            E_tr = eop("pe", tr, waits=[
                (f"dma_sig{pb}", E_sig_g), ("pool", E_ident),
                (ceng, slot_last[s])])

            def cp(b=b, bank=bank, ceng=ceng):
                op = (nc.scalar.copy if ceng == "act"
                      else nc.vector.tensor_copy)
                return op(sig_ap(b, 1, 0), bank[:, 0:128])
            slot_last[s] = eop(ceng, cp, waits=[("pe", E_tr)])
        E_bt_act, E_bt_dve = slot_last[0], slot_last[1]

        E_cp = 0
        for r in range(npow_):
            w = [("pool", E_ident), ("dve", max(E_zT0, E_bt_dve)),
                 ("act", max(E_cp, E_bt_act))]
            _, _, E_mm = matvec(w, b_range=range(b0, b0 + POW_BATCH))
            if r < npow_ - 1:
                def cp0(b0=b0):
                    return nc.scalar.copy(
                        zT[:].rearrange("p (h b) -> p h b", h=H)[
                            :, 0, b0:b0 + POW_BATCH],
                        PBy[:, b0:b0 + POW_BATCH])

                def cp1(b0=b0):
                    return nc.scalar.copy(
                        zT[:].rearrange("p (h b) -> p h b", h=H)[
                            :, 1, b0:b0 + POW_BATCH],
                        PBy1[:, b0:b0 + POW_BATCH])
                eop("act", cp0, waits=[("pe", E_mm)])
                E_cp = eop("act", cp1)
            else:
                E_pmm_last = E_mm

    # --------------------------------------------------------- Rayleigh
    eop("dve", lambda: nc.vector.tensor_tensor(
        pq[:, 0:64], PBy[:, 0:64], zT[:, 0:64], ALU.mult),
        waits=[("pe", E_pmm_last)])
    E_pq = eop("dve", lambda: nc.vector.tensor_tensor(
        pq[:, 64:128], PBy1[:, 0:64], zT[:, 64:128], ALU.mult))
    E_qq = eop("dve", lambda: nc.vector.tensor_tensor(
        qq[:], zT[:], zT[:], ALU.mult), waits=[("dve", E_pq)])
    eop("pe", lambda: nc.tensor.matmul(
        PBr[0:1, 0:128], ones[:], pq[:], start=True, stop=True),
        waits=[("dve", E_qq)])
    E_red = eop("pe", lambda: nc.tensor.matmul(
        PBr[0:1, 128:256], ones[:], qq[:], start=True, stop=True))

    # t=0 state init (zT reused: re-memset after qq consumed it)
    E_zTi = eop("dve", lambda: nc.vector.memset(zT[:], 1.0 / P),
                waits=[("dve", E_qq)])
    E_zri = eop("dve", lambda: nc.vector.memset(zrow[:], 1.0 / P))
    E_wAi = eop("dve", lambda: nc.vector.memset(wA[:], 1.0 / P))
    eop("dve", lambda: nc.vector.memset(tau[:], 0.0))
    E_tau = eop("dve", lambda: nc.vector.memset(X64[:], -1.0))

    # t=0 matvec (emitted early; PE works while DVE does the scalar chain)
    E_mm0_h0, E_mm0_mid, E_mm0 = matvec([("dve", E_zTi)], tick_each_hi=True)

    # ------------------------------------------------------ scalar chain
    TS = nc.vector.tensor_scalar
    TT = nc.vector.tensor_tensor
    STT = nc.vector.scalar_tensor_tensor
    num, den, rden, R, L = scx(0), scx(1), scx(2), scx(3), scx(4)
    sq, onep, rop, onem = scx(5), scx(6), scx(7), scx(8)
    stp_r, ev_r, m2a_r, dv_r, th_r, th1_r = (scx(9), scx(10), scx(11),
                                             scx(12), scx(13), scx(14))

    prs = SCX[0:1, 768:1024]   # SBUF copy of the PE-reduce results
    e = eop("dve", lambda: nc.vector.tensor_copy(prs, PBr[0:1, 0:256]),
            waits=[("pe", E_red)])
    e = eop("dve", lambda: TT(num, SCX[0:1, 768:832], SCX[0:1, 832:896],
                              ALU.add), waits=[("dve", e)])
    e = eop("dve", lambda: TT(den, SCX[0:1, 896:960], SCX[0:1, 960:1024],
                              ALU.add), waits=[("dve", e)])
    e = eop("dve", lambda: TS(den, den, EPS, None, ALU.add),
            waits=[("dve", e)])
    e = eop("dve", lambda: nc.vector.reciprocal(rden, den),
            waits=[("dve", e)])
    e = eop("dve", lambda: TT(R, num, rden, ALU.mult), waits=[("dve", e)])
    e = eop("dve", lambda: TS(L, R, 2.0 * SAFETY, 2.0 * lam2 + 1e-6,
                              ALU.mult, ALU.add), waits=[("dve", e)])
    e = eop("dve", lambda: nc.vector.reciprocal(stp_r, L),
            waits=[("dve", e)])
    e = eop("dve", lambda: TS(ev_r, stp_r, -2.0 * lam2, 1.0, ALU.mult,
                              ALU.add), waits=[("dve", e)])
    e = eop("dve", lambda: TS(m2a_r, stp_r, -2.0, None, ALU.mult),
            waits=[("dve", e)])
    E_dv = eop("dve", lambda: TS(dv_r, stp_r, 2.0 * lam2, None, ALU.mult),
               waits=[("dve", e)])
    E_sq = eop("act", lambda: nc.scalar.activation(sq, dv_r, ACTF.Sqrt),
               waits=[("dve", E_dv)])
    e = eop("dve", lambda: TS(onep, sq, 1.0, None, ALU.add),
            waits=[("act", E_sq)])
    e = eop("dve", lambda: nc.vector.reciprocal(rop, onep),
            waits=[("dve", e)])
    e = eop("dve", lambda: TS(onem, sq, -1.0, 1.0, ALU.mult, ALU.add),
            waits=[("dve", e)])
    e = eop("dve", lambda: TT(th_r, onem, rop, ALU.mult),
            waits=[("dve", e)])
    E_scp = eop("dve", lambda: TS(th1_r, th_r, 1.0, None, ALU.add),
                waits=[("dve", e)])
    # transpose each [1,64] scalar row -> [64,1]; land in PBzt cols 0..5
    scalar_rows = [stp_r, ev_r, m2a_r, dv_r, th_r, th1_r]
    for k, row in enumerate(scalar_rows):
        def mk(k=k, row=row):
            return nc.tensor.transpose(PBzt[0:64, k:k + 1], row,
                                       ident[0:1, 0:1])
        if k == 0:
            eop("pe", mk, waits=[("dve", E_scp), ("act", slot_last[0])],
                inc=False)
        elif k == len(scalar_rows) - 1:
            E_sctr = eop("pe", mk)
        else:
            mk()
    E_sct = eop("act", lambda: nc.scalar.copy(SCT[0:64, 0:6],
                                              PBzt[0:64, 0:6]),
                waits=[("pe", E_sctr)])

    # fv = stp*(beta - lam1) + (2*lam2*stp)*w_prev   (f16 out)
    e = eop("dve", lambda: TS(fvw[:], wprow[:], DV, None, ALU.mult),
            waits=[("act", E_sct), ("dma_bw", E_bw)])
    e = eop("dve", lambda: TS(fv[:], brow[:], lam1, None, ALU.subtract),
            waits=[("dve", e)])
    E_fv = eop("dve", lambda: STT(fv[:], fv[:], STP, fvw[:], ALU.mult,
                                  ALU.add), waits=[("dve", e)])
    # ws0 = th * w0
    E_ws = eop("dve", lambda: TS(ws[:], wA[:], TH, None, ALU.mult),
               waits=[("dve", max(E_fv, E_wAi))])

    # ---------------------------------------------------------- FISTA
    E_zTcopy = 0
    E_v = 0                       # PByr WAR gate for ytr
    E_z = E_zri
    E_u = 0
    E_v = 0

    for t in range(T):
        wold = wA if t % 2 == 0 else wB
        wnew = wB if t % 2 == 0 else wA
        last = t == T - 1

        if t > 0:
            # ztr per half: ztr0 gated only on z-h0 so it overlaps z-h1
            gate = (("act", E_sct) if t == 1 else ("dve", E_zTcopy))
            E_ztr0 = eop("pe", lambda: nc.tensor.transpose(
                PBzt[:, 0:64], zrow[:, 0:128], ident[0:64, 0:64]),
                waits=[("dve", E_z), gate])
            E_ztr = eop("pe", lambda: nc.tensor.transpose(
                PBzt[:, 64:128], zrow[:, 128:256], ident[0:64, 0:64]))
            E_zTcopy = eop("dve", lambda: nc.vector.tensor_copy(
                zT[:], PBzt[:, 0:128]), waits=[("pe", E_ztr)])
            E_mm_h0, E_mm_mid, E_mm = matvec([("dve", E_zTcopy)],
                                             tick_each_hi=True)
        else:
            E_mm_h0, E_mm_mid, E_mm = E_mm0_h0, E_mm0_mid, E_mm0
        # u = ev*z + fv on DVE, scheduled here so it runs under the PE
        # matvec (reads z from the previous iteration's tail)
        E_u = eop("dve", lambda: STT(u[:], zrow[:], EV, fv[:],
                                     ALU.mult, ALU.add),
                  waits=[("dve", max(E_z, E_fv, E_v))])

        # per-half: copy PBy/PBy1 -> ycol (DVE), transpose to PByr (PE):
        # half hi lands at PByr[0:64, hi*128:(hi+1)*128] (row layout).
        E_yc0 = eop("dve", lambda: nc.vector.tensor_copy(
            ycol[:, 0:64], PBy[:, 0:64]), waits=[("pe", E_mm_h0)])
        E_ytr0 = eop("pe", lambda: nc.tensor.transpose(
            PByr[0:64, 0:128], ycol[:, 0:64], ident[:]),
            waits=[("dve", max(E_yc0, E_v))])
        eop("dve", lambda: nc.vector.tensor_copy(
            ycol[:, 64:96], PBy1[:, 0:32]), waits=[("pe", E_mm_mid)])
        E_yc1 = eop("dve", lambda: nc.vector.tensor_copy(
            ycol[:, 96:128], PBy1[:, 32:64]), waits=[("pe", E_mm)])
        E_ytr = eop("pe", lambda: nc.tensor.transpose(
            PBr[0:64, 0:128], ycol[:, 64:128], ident[:]),
            waits=[("dve", E_yc1)])

        # v = m2a*y + u (f16 out), split: v-h0 runs under yc1/ytr1
        eop("dve", lambda: STT(vrow[:, 0:128], PByr[0:64, 0:128], M2A,
                               u[:, 0:128], ALU.mult, ALU.add),
            waits=[("pe", E_ytr0), ("dve", E_u)])
        E_v = eop("dve", lambda: STT(vrow[:, 128:256], PBr[0:64, 0:128],
                                     M2A, u[:, 128:256], ALU.mult,
                                     ALU.add),
                  waits=[("pe", E_ytr)])

        # Newton: phi = sum max(v,tau) - P*tau - 1; cnt = #(v>tau) + 1
        # X64 = -P*tau - 1 is precomputed off the critical path (end of
        # the previous iteration / inner step), so the v->tau chain is
        # S1,cnt,G,rc,tau.
        nsteps = N0 if t == 0 else 1
        if t == 0:
            # analytic warm start: tau0 = (sum(v) - 1)/P
            e0 = eop("dve", lambda: TS(dumA[:], vrow[:], 0.0, None,
                                       ALU.add, ALU.add,
                                       accum_out=S1a[:]),
                     waits=[("dve", max(E_v, E_tau))])
            e0 = eop("dve", lambda: TS(tau[:], S1a[:], 1.0 / P,
                                       -1.0 / P, ALU.mult, ALU.add),
                     waits=[("dve", e0)])
            E_tau = eop("dve", lambda: TS(X64[:], tau[:], -float(P),
                                          -1.0, ALU.mult, ALU.add),
                        waits=[("dve", e0)])
        for k in range(nsteps):
            e1 = eop("dve", lambda: TS(dumA[:], vrow[:], tau[:], None,
                                       ALU.max, ALU.add, accum_out=S1a[:]),
                     waits=[("dve", max(E_v, E_tau))])
            e2 = eop("dve", lambda: TS(dumB[:], vrow[:], tau[:], None,
                                       ALU.is_gt, ALU.add,
                                       accum_out=cnta[:]),
                     waits=[("dve", e1)])
            e = eop("dve", lambda: TT(G64[:], X64[:], S1a[:], ALU.add),
                    waits=[("dve", e2)])
            e = eop("dve", lambda: nc.vector.reciprocal(rc[:], cnta[:]),
                    waits=[("dve", e)])
            E_tau = eop("dve", lambda: STT(tau[:], G64[:], rc[:], tau[:],
                                           ALU.mult, ALU.add),
                        waits=[("dve", e)])
            if k < nsteps - 1:
                E_tau = eop("dve", lambda: TS(X64[:], tau[:], -float(P),
                                              -1.0, ALU.mult, ALU.add),
                            waits=[("dve", E_tau)])

        # w = clip(v - tau, 0, MAX_W)
        e = eop("dve", lambda: TS(t1[:], vrow[:], tau[:], 0.0,
                                  ALU.subtract, ALU.max),
                waits=[("dve", E_tau)])
        E_wn = eop("dve", lambda: TS(wnew[:], t1[:], MAX_W, None, ALU.min),
                   waits=[("dve", e)])

        if not last:
            E_zh0 = 0
            E_z = eop("dve", lambda: STT(zrow[:], wnew[:], TH1, ws[:],
                                         ALU.mult, ALU.subtract),
                      waits=[("dve", max(E_wn, E_ws))])
            E_zh0 = E_z
            E_ws = eop("dve", lambda: TS(ws[:], wnew[:], TH, None,
                                        ALU.mult), waits=[("dve", E_z)])
            eop("dve", lambda: TS(X64[:], tau[:], -float(P), -1.0,
                                  ALU.mult, ALU.add),
                waits=[("dve", E_ws)])
        else:
            # renormalize: out = w / (sum(w) + EPS)
            e = eop("dve", lambda: TS(dumA[:], wnew[:], 0.0, None,
                                      ALU.add, ALU.add,
                                      accum_out=ssum[:]),
                    waits=[("dve", E_wn)])
            e = eop("dve", lambda: TS(ssum[:], ssum[:], EPS, None,
                                      ALU.add), waits=[("dve", e)])
            e = eop("dve", lambda: nc.vector.reciprocal(rs[:], ssum[:]),
                    waits=[("dve", e)])
            E_out = eop("dve", lambda: TS(outt[:], wnew[:], rs[:], None,
                                          ALU.mult), waits=[("dve", e)])

    # ---------------------------------------------------------- store
    nc.sync.wait_ge(sems["dve"], E_out)
    d = nc.sync.dma_start(out=out_d, in_=outt[:])
    d.then_inc(sems["dma_out"], 16)
    nc.sync.wait_ge(sems["dma_out"], 16)


def build(lam1, lam2, T=None, npow=None):
    nc = bass.Bass("TRN2", target_bir_lowering=False, debug=False)
    sigma_d = nc.dram_tensor("sigma", [NB, P, P], F32, kind="ExternalInput")
    beta_d = nc.dram_tensor("beta", [NB, P], F32, kind="ExternalInput")
    wprev_d = nc.dram_tensor("w_prev", [NB, P], F32, kind="ExternalInput")
    out_d = nc.dram_tensor("out", [NB, P], F32, kind="ExternalOutput")
    with ExitStack() as ctx:
        _emit(ctx, nc, sigma_d.ap(), beta_d.ap(), wprev_d.ap(), out_d.ap(),
              lam1, lam2, T=T, npow=npow)
    return nc


def kernel(sigma, beta, w_prev, log_lambda1, log_lambda2):
    global LAST_RESULT
    sigma = np.ascontiguousarray(np.asarray(sigma, dtype=np.float32))
    beta = np.ascontiguousarray(np.asarray(beta, dtype=np.float32))
    w_prev = np.ascontiguousarray(np.asarray(w_prev, dtype=np.float32))
    lam1 = float(np.exp(np.float32(log_lambda1)))
    lam2 = float(np.exp(np.float32(log_lambda2)))

    nc = build(lam1, lam2)
    in_maps = []
    for c in range(N_CORES):
        s = slice(c * NB, (c + 1) * NB)
        in_maps.append({
            "sigma": sigma[s],
            "beta": beta[s],
            "w_prev": w_prev[s],
        })
    res = run_bass_kernel_spmd(nc, in_maps, list(range(N_CORES)), trace=TRACE)
    LAST_RESULT = res
    out = np.concatenate([res.results[c]["out"] for c in range(N_CORES)],
                         axis=0)
    return np.ascontiguousarray(out.astype(np.float32))


# revision 41
# speedup vs baseline: 11.9171x; 1.0328x over previous
"""Trainium2 Bass kernel for nn_DifferentiableRiskBudgeting.

Per batch sample b (data parallel, 64 samples per core on 8 cores):
    min_w  w' S_b w - beta_b' w + lam1*||w||_1 + lam2*||w - w_prev||^2
    s.t.   sum w = 1, 0 <= w <= MAX_W
then clamp + renormalize (matching the reference's converged 250-step
PGD fixed point; the QP is strongly convex so the fixed point is
unique).

Algorithm (numpy-validated, rel err ~2.8e-3 vs reference incl. fp16
iterate storage, gate 2e-2):
  - NPOW unnormalized power rounds z <- S z, then Rayleigh quotient
    R = (z3.z2)/(z2.z2) and per-sample step 1/(2*SAFETY*R + 2*lam2).
    FISTA tolerates steps up to 2/L so SAFETY=1.15 on the power
    underestimate is safe (validated worst-case underestimate 1.6x).
  - FISTA with per-sample momentum theta=(1-sqrt(q))/(1+sqrt(q)),
    q = 2*lam2*step, and the scaled-momentum state ws = theta*w so the
    critical-path z update is a single scalar_tensor_tensor:
        z = (1+theta)*w_new - ws;   ws = theta*w_new (off-path)
  - Simplex projection via ONE warm damped-Newton step per iteration
    (N0 cold steps on the first), ignoring the w<=MAX_W cap in the tau
    solve (cap binds for ~3 of 512 samples; the clip + final
    renormalize absorb it — validated):
        phi  = sum max(v,tau) - P*tau - 1
        cnt  = #(v > tau) + 1          (damped slope)
        tau += phi / cnt

Layouts: row [64, 256] (partition = sample, free = feature) for all
elementwise work, so per-sample scalars are per-partition APs and
tensor_scalar accumulations give full per-sample sums directly (walrus
requires matching partition ranges on elementwise operands and
PSUM access patterns starting at partition 0 — no cross-partition
folds are legal).  Column [128, 128] (partition = j within half hj,
free = hj*64 + b) for the PE matvec; sigma symmetric so row-major
chunks double as the transposed stationary operand.

Sigma DMA exploits symmetry: only rows 0:128 (A|B) and the C block are
loaded (75% of bytes); the B^T chunk is reconstructed on-device by PE
transposes + ACT/DVE copies hidden under the DMA (~38us at the modeled
360 GB/s).  Power rounds also pipeline per 8-sample batch under the
DMA, so the spectral estimate is free.

Iterate tensors are fp16 (DVE 4x mode on the clip/reduction passes);
the matvec path (sigma, zT, PSUM) stays fp32.  The y halves land in
two separate PSUM banks so a consumer read of half 0 never shares a
bank with the in-flight PE writes of half 1 (same-bank PE-write +
engine-read is fatal on HW).

Raw bass with explicit semaphores; waits are FUSED onto the consuming
instruction (1 wait + 1 inc <= walrus' ~2 sync commands per
instruction), extra waits standalone.  Same-engine dependent ops also
need inc+wait pairs (engine pipelines do not interlock); ordering is
transitive through any later same-engine inc.
"""

import math
import numpy as np
from contextlib import ExitStack

import concourse.bass as bass
from concourse import mybir
from concourse.bass_utils import run_bass_kernel_spmd

F32 = mybir.dt.float32
F16 = mybir.dt.float16
ALU = mybir.AluOpType
ACTF = mybir.ActivationFunctionType

B, P = 512, 256
N_CORES = 8
NB = B // N_CORES            # samples per core (64)
H = 2                        # feature halves
MAX_W = 0.1
EPS = 1e-8

NPOW = 2                     # unnormalized power rounds
SAFETY = 1.3
T_FISTA = 17
N0 = 2                       # Newton steps, first iteration
SIG_DMA_BATCH = 4            # samples per sigma DMA
POW_BATCH = 8                # samples per pipelined power batch

TRACE = False
LAST_RESULT = None


def _emit(ctx, nc, sigma_d, beta_d, wprev_d, out_d, lam1, lam2,
          T=None, npow=None):
    nb = NB
    T = T_FISTA if T is None else T
    npow_ = NPOW if npow is None else npow

    def sbuf(name, shape, dt=F32):
        return ctx.enter_context(nc.sbuf_tensor(name, shape, dt))

    def psum(name):
        return ctx.enter_context(nc.psum_tensor(name, [128, 512], F32))

    ENG = {"pe": nc.tensor, "dve": nc.vector, "act": nc.scalar,
           "pool": nc.gpsimd, "sync": nc.sync}
    sems = {e: ctx.enter_context(nc.semaphore(f"s_{e}"))
            for e in ("pe", "dve", "act", "pool", "dma_bw", "dma_out")}
    npb = NB // POW_BATCH
    for g in range(npb):
        sems[f"dma_sig{g}"] = ctx.enter_context(nc.semaphore(f"s_dsig{g}"))
    ctr = {e: 0 for e in sems}
    last_wait = {e: {} for e in list(ENG)}

    def eop(eng, emit, waits=(), inc=True):
        """Emit an op on `eng` with fused sync (see module docstring)."""
        lw = last_wait[eng]
        pend = []
        for s, v in waits:
            if v and v > 0 and lw.get(s, 0) < v:
                pend.append((s, v))
                lw[s] = v
        for s, v in pend[1:]:
            ENG[eng].wait_ge(sems[s], v)
        inst = emit()
        if pend:
            s, v = pend[0]
            inst.wait_op(sems[s], v, "sem-ge")
        if inc:
            ctr[eng] += 1
            inst.then_inc(sems[eng], 1)
            return ctr[eng]
        return None

    # ------------------------------------------------------------- tensors
    nbatch = SIG_DMA_BATCH
    nk = nb // nbatch
    sig = [sbuf(f"sig{k}", [128, nbatch * H * P]) for k in range(nk)]

    def sig_ap(b, hj, hi):
        k, m = divmod(b, nbatch)
        c0 = (m * H + hj) * P + hi * 128
        return sig[k][:, c0:c0 + 128]

    ident = sbuf("ident", [128, 128])
    ones = sbuf("ones", [128, 1])
    zT = sbuf("zT", [128, 128])          # col: [j-in-half, hj*64+b], f32
    ycol = sbuf("ycol", [128, 128])      # col: [i-in-half, hi*64+b], f32
    pq = sbuf("pq", [128, 128])
    qq = sbuf("qq", [128, 128])
    zrow = sbuf("zrow", [64, 256])       # row: [b, j]
    u = sbuf("u", [64, 256])
    fv = sbuf("fv", [64, 256])
    fvw = sbuf("fvw", [64, 256])
    vrow = sbuf("vrow", [64, 256], F16)
    t1 = sbuf("t1", [64, 256], F16)
    wA = sbuf("wA", [64, 256], F16)
    wB = sbuf("wB", [64, 256], F16)
    ws = sbuf("ws", [64, 256], F16)
    brow = sbuf("brow", [64, 256])
    wprow = sbuf("wprow", [64, 256])
    outt = sbuf("outt", [64, 256])
    dumA = sbuf("dumA", [64, 256], F16)
    dumB = sbuf("dumB", [64, 256], F16)
    SCT = sbuf("SCT", [64, 8])           # cols: stp, ev, m2a, dv, th, th1
    SCX = sbuf("SCX", [1, 1024])         # [1,64] scratch slices
    tau = sbuf("tau", [64, 1])
    S1a = sbuf("S1a", [64, 1])
    cnta = sbuf("cnta", [64, 1])
    X64 = sbuf("X64", [64, 1])
    G64 = sbuf("G64", [64, 1])
    rc = sbuf("rc", [64, 1])
    ssum = sbuf("ssum", [64, 1])
    rs = sbuf("rs", [64, 1])

    PBzt = psum("PBzt")
    PBy = psum("PBy")
    PBy1 = psum("PBy1")      # hi=1 matvec bank (bank-isolated from PBy)
    PByr = psum("PByr")
    PBr = psum("PBr")

    STP = SCT[:, 0:1]
    EV = SCT[:, 1:2]
    M2A = SCT[:, 2:3]
    DV = SCT[:, 3:4]
    TH = SCT[:, 4:5]
    TH1 = SCT[:, 5:6]

    def scx(k):
        return SCX[0:1, 64 * k:64 * (k + 1)]

    # --------------------------------------------------------------- DMAs
    # sigma symmetric: load A|B rows + C block; B^T rebuilt on-device.
    kb_per_g = nk // npb
    for k in range(nk):
        ks = sigma_d[k * nbatch:(k + 1) * nbatch]
        dst4 = sig[k][:].rearrange("p (b h j) -> p b h j", b=nbatch, h=H)
        d = nc.sync.dma_start(
            out=dst4[:, :, 0, :],
            in_=ks[:, 0:128, :].rearrange("b p j -> p b j"))
        d.then_inc(sems[f"dma_sig{k // kb_per_g}"], 16)
        d = nc.sync.dma_start(
            out=dst4[:, :, 1, 128:256],
            in_=ks[:, 128:256, 128:256].rearrange("b p j -> p b j"))
        d.then_inc(sems[f"dma_sig{k // kb_per_g}"], 16)
    E_sig_g = 32 * kb_per_g
    d = nc.sync.dma_start(out=brow[:], in_=beta_d)
    d.then_inc(sems["dma_bw"], 16)
    d = nc.sync.dma_start(out=wprow[:], in_=wprev_d)
    d.then_inc(sems["dma_bw"], 16)
    E_bw = 32

    # ------------------------------------------------------------ preamble
    eop("dve", lambda: nc.vector.memset(ident[:], 0.0))
    eop("dve", lambda: nc.vector.memset(ones[:], 1.0))
    E_zT0 = eop("dve", lambda: nc.vector.memset(zT[:], 1.0 / math.sqrt(P)))
    E_ident = eop("pool", lambda: nc.gpsimd.affine_select(
        out=ident[:], in_=ident[:], compare_op=ALU.not_equal, fill=1.0,
        base=0, pattern=[[-1, 128]], channel_multiplier=1),
        waits=[("dve", E_zT0)])

    def matvec(gate_waits, b_range=range(nb), tick_each_hi=False):
        """Column-space matvec: y-half hi of sample b -> (PBy|PBy1)[:, b].

        Returns (tick_hi0, tick_hi1_mid, tick_all) pe ticks."""
        t_hi0 = t_mid = None
        bmid = b_range[len(b_range) // 2 - 1]
        first = True
        for hi in range(H):
            dst = PBy if hi == 0 else PBy1
            for b in b_range:
                for hj in range(H):
                    def mk(b=b, hj=hj, hi=hi, dst=dst):
                        return nc.tensor.matmul(
                            dst[:, b:b + 1], sig_ap(b, hj, hi),
                            zT[:, hj * 64 + b:hj * 64 + b + 1],
                            start=(hj == 0), stop=(hj == H - 1))
                    is_last = (b == b_range[-1] and hj == H - 1)
                    is_mid = (hi == H - 1 and b == bmid and hj == H - 1)
                    if first:
                        tick = eop("pe", mk, waits=gate_waits)
                        first = False
                    elif is_last and (hi == H - 1 or tick_each_hi):
                        tick = eop("pe", mk)
                    elif is_mid and tick_each_hi:
                        t_mid = eop("pe", mk)
                    else:
                        mk()
            if hi == 0:
                t_hi0 = tick if tick_each_hi else None
        return t_hi0, t_mid, tick

    # ------------------------------- B^T reconstruction + power (pipelined)
    E_pmm_last = 0
    slot_last = {0: 0, 1: 0}          # last B-copy tick per slot (act/dve)
    for pb in range(npb):
        b0 = pb * POW_BATCH
        for bb in range(POW_BATCH):
            b = b0 + bb
            s = bb % 2
            bank = PBzt if s == 0 else PBr
            ceng = "act" if s == 0 else "dve"

            def tr(b=b, bank=bank):
                return nc.tensor.transpose(bank[:, 0:128],
                                           sig_ap(b, 0, 1), ident[:])
            E_tr = eop("pe", tr, waits=[
                (f"dma_sig{pb}", E_sig_g), ("pool", E_ident),
                (ceng, slot_last[s])])

            def cp(b=b, bank=bank, ceng=ceng):
                op = (nc.scalar.copy if ceng == "act"
                      else nc.vector.tensor_copy)
                return op(sig_ap(b, 1, 0), bank[:, 0:128])
            slot_last[s] = eop(ceng, cp, waits=[("pe", E_tr)])
        E_bt_act, E_bt_dve = slot_last[0], slot_last[1]

        E_cp = 0
        for r in range(npow_):
            w = [("pool", E_ident), ("dve", max(E_zT0, E_bt_dve)),
                 ("act", max(E_cp, E_bt_act))]
            _, _, E_mm = matvec(w, b_range=range(b0, b0 + POW_BATCH))
            if r < npow_ - 1:
                def cp0(b0=b0):
                    return nc.scalar.copy(
                        zT[:].rearrange("p (h b) -> p h b", h=H)[
                            :, 0, b0:b0 + POW_BATCH],
                        PBy[:, b0:b0 + POW_BATCH])

                def cp1(b0=b0):
                    return nc.scalar.copy(
                        zT[:].rearrange("p (h b) -> p h b", h=H)[
                            :, 1, b0:b0 + POW_BATCH],
                        PBy1[:, b0:b0 + POW_BATCH])
                eop("act", cp0, waits=[("pe", E_mm)])
                E_cp = eop("act", cp1)
            else:
                E_pmm_last = E_mm

    # --------------------------------------------------------- Rayleigh
    eop("dve", lambda: nc.vector.tensor_tensor(
        pq[:, 0:64], PBy[:, 0:64], zT[:, 0:64], ALU.mult),
        waits=[("pe", E_pmm_last)])
    E_pq = eop("dve", lambda: nc.vector.tensor_tensor(
        pq[:, 64:128], PBy1[:, 0:64], zT[:, 64:128], ALU.mult))
    E_qq = eop("dve", lambda: nc.vector.tensor_tensor(
        qq[:], zT[:], zT[:], ALU.mult), waits=[("dve", E_pq)])
    eop("pe", lambda: nc.tensor.matmul(
        PBr[0:1, 0:128], ones[:], pq[:], start=True, stop=True),
        waits=[("dve", E_qq)])
    E_red = eop("pe", lambda: nc.tensor.matmul(
        PBr[0:1, 128:256], ones[:], qq[:], start=True, stop=True))

    # t=0 state init (zT reused: re-memset after qq consumed it)
    E_zTi = eop("dve", lambda: nc.vector.memset(zT[:], 1.0 / P),
                waits=[("dve", E_qq)])
    E_zri = eop("dve", lambda: nc.vector.memset(zrow[:], 1.0 / P))
    E_wAi = eop("dve", lambda: nc.vector.memset(wA[:], 1.0 / P))
    eop("dve", lambda: nc.vector.memset(tau[:], 0.0))
    E_tau = eop("dve", lambda: nc.vector.memset(X64[:], -1.0))

    # t=0 matvec (emitted early; PE works while DVE does the scalar chain)
    E_mm0_h0, E_mm0_mid, E_mm0 = matvec([("dve", E_zTi)], tick_each_hi=True)

    # ------------------------------------------------------ scalar chain
    TS = nc.vector.tensor_scalar
    TT = nc.vector.tensor_tensor
    STT = nc.vector.scalar_tensor_tensor
    num, den, rden, R, L = scx(0), scx(1), scx(2), scx(3), scx(4)
    sq, onep, rop, onem = scx(5), scx(6), scx(7), scx(8)
    stp_r, ev_r, m2a_r, dv_r, th_r, th1_r = (scx(9), scx(10), scx(11),
                                             scx(12), scx(13), scx(14))

    prs = SCX[0:1, 768:1024]   # SBUF copy of the PE-reduce results
    e = eop("dve", lambda: nc.vector.tensor_copy(prs, PBr[0:1, 0:256]),
            waits=[("pe", E_red)])
    e = eop("dve", lambda: TT(num, SCX[0:1, 768:832], SCX[0:1, 832:896],
                              ALU.add), waits=[("dve", e)])
    e = eop("dve", lambda: TT(den, SCX[0:1, 896:960], SCX[0:1, 960:1024],
                              ALU.add), waits=[("dve", e)])
    e = eop("dve", lambda: TS(den, den, EPS, None, ALU.add),
            waits=[("dve", e)])
    e = eop("dve", lambda: nc.vector.reciprocal(rden, den),
            waits=[("dve", e)])
    e = eop("dve", lambda: TT(R, num, rden, ALU.mult), waits=[("dve", e)])
    e = eop("dve", lambda: TS(L, R, 2.0 * SAFETY, 2.0 * lam2 + 1e-6,
                              ALU.mult, ALU.add), waits=[("dve", e)])
    e = eop("dve", lambda: nc.vector.reciprocal(stp_r, L),
            waits=[("dve", e)])
    e = eop("dve", lambda: TS(ev_r, stp_r, -2.0 * lam2, 1.0, ALU.mult,
                              ALU.add), waits=[("dve", e)])
    e = eop("dve", lambda: TS(m2a_r, stp_r, -2.0, None, ALU.mult),
            waits=[("dve", e)])
    E_dv = eop("dve", lambda: TS(dv_r, stp_r, 2.0 * lam2, None, ALU.mult),
               waits=[("dve", e)])
    E_sq = eop("act", lambda: nc.scalar.activation(sq, dv_r, ACTF.Sqrt),
               waits=[("dve", E_dv)])
    e = eop("dve", lambda: TS(onep, sq, 1.0, None, ALU.add),
            waits=[("act", E_sq)])
    e = eop("dve", lambda: nc.vector.reciprocal(rop, onep),
            waits=[("dve", e)])
    e = eop("dve", lambda: TS(onem, sq, -1.0, 1.0, ALU.mult, ALU.add),
            waits=[("dve", e)])
    e = eop("dve", lambda: TT(th_r, onem, rop, ALU.mult),
            waits=[("dve", e)])
    E_scp = eop("dve", lambda: TS(th1_r, th_r, 1.0, None, ALU.add),
                waits=[("dve", e)])
    # transpose each [1,64] scalar row -> [64,1]; land in PBzt cols 0..5
    scalar_rows = [stp_r, ev_r, m2a_r, dv_r, th_r, th1_r]
    for k, row in enumerate(scalar_rows):
        def mk(k=k, row=row):
            return nc.tensor.transpose(PBzt[0:64, k:k + 1], row,
                                       ident[0:1, 0:1])
        if k == 0:
            eop("pe", mk, waits=[("dve", E_scp), ("act", slot_last[0])],
                inc=False)
        elif k == len(scalar_rows) - 1:
            E_sctr = eop("pe", mk)
        else:
            mk()
    E_sct = eop("act", lambda: nc.scalar.copy(SCT[0:64, 0:6],
                                              PBzt[0:64, 0:6]),
                waits=[("pe", E_sctr)])

    # fv = stp*(beta - lam1) + (2*lam2*stp)*w_prev   (f16 out)
    e = eop("dve", lambda: TS(fvw[:], wprow[:], DV, None, ALU.mult),
            waits=[("act", E_sct), ("dma_bw", E_bw)])
    e = eop("dve", lambda: TS(fv[:], brow[:], lam1, None, ALU.subtract),
            waits=[("dve", e)])
    E_fv = eop("dve", lambda: STT(fv[:], fv[:], STP, fvw[:], ALU.mult,
                                  ALU.add), waits=[("dve", e)])
    # ws0 = th * w0
    E_ws = eop("dve", lambda: TS(ws[:], wA[:], TH, None, ALU.mult),
               waits=[("dve", max(E_fv, E_wAi))])

    # ---------------------------------------------------------- FISTA
    E_zTcopy = 0
    E_v = 0                       # PByr WAR gate for ytr
    E_z = E_zri
    E_u = 0
    E_v = 0

    for t in range(T):
        wold = wA if t % 2 == 0 else wB
        wnew = wB if t % 2 == 0 else wA
        last = t == T - 1

        if t > 0:
            # ztr per half: ztr0 gated only on z-h0 so it overlaps z-h1
            gate = (("act", E_sct) if t == 1 else ("dve", E_zTcopy))
            E_ztr0 = eop("pe", lambda: nc.tensor.transpose(
                PBzt[:, 0:64], zrow[:, 0:128], ident[0:64, 0:64]),
                waits=[("dve", E_z), gate])
            E_ztr = eop("pe", lambda: nc.tensor.transpose(
                PBzt[:, 64:128], zrow[:, 128:256], ident[0:64, 0:64]))
            E_zTcopy = eop("dve", lambda: nc.vector.tensor_copy(
                zT[:], PBzt[:, 0:128]), waits=[("pe", E_ztr)])
            E_mm_h0, E_mm_mid, E_mm = matvec([("dve", E_zTcopy)],
                                             tick_each_hi=True)
        else:
            E_mm_h0, E_mm_mid, E_mm = E_mm0_h0, E_mm0_mid, E_mm0
        # u = ev*z + fv on DVE, scheduled here so it runs under the PE
        # matvec (reads z from the previous iteration's tail)
        E_u = eop("dve", lambda: STT(u[:], zrow[:], EV, fv[:],
                                     ALU.mult, ALU.add),
                  waits=[("dve", max(E_z, E_fv, E_v))])

        # per-half: copy PBy/PBy1 -> ycol (DVE), transpose to PByr (PE):
        # half hi lands at PByr[0:64, hi*128:(hi+1)*128] (row layout).
        E_yc0 = eop("dve", lambda: nc.vector.tensor_copy(
            ycol[:, 0:64], PBy[:, 0:64]), waits=[("pe", E_mm_h0)])
        E_ytr0 = eop("pe", lambda: nc.tensor.transpose(
            PByr[0:64, 0:128], ycol[:, 0:64], ident[:]),
            waits=[("dve", max(E_yc0, E_v))])
        eop("dve", lambda: nc.vector.tensor_copy(
            ycol[:, 64:96], PBy1[:, 0:32]), waits=[("pe", E_mm_mid)])
        E_yc1 = eop("dve", lambda: nc.vector.tensor_copy(
            ycol[:, 96:128], PBy1[:, 32:64]), waits=[("pe", E_mm)])
        E_ytr = eop("pe", lambda: nc.tensor.transpose(
            PBr[0:64, 0:128], ycol[:, 64:128], ident[:]),
            waits=[("dve", E_yc1)])

        # v = m2a*y + u (f16 out), split: v-h0 runs under yc1/ytr1
        eop("dve", lambda: STT(vrow[:, 0:128], PByr[0:64, 0:128], M2A,
                               u[:, 0:128], ALU.mult, ALU.add),
            waits=[("pe", E_ytr0), ("dve", E_u)])
        E_v = eop("dve", lambda: STT(vrow[:, 128:256], PBr[0:64, 0:128],
                                     M2A, u[:, 128:256], ALU.mult,
                                     ALU.add),
                  waits=[("pe", E_ytr)])

        # Newton: phi = sum max(v,tau) - P*tau - 1; cnt = #(v>tau) + 1
        # X64 = -P*tau - 1 is precomputed off the critical path (end of
        # the previous iteration / inner step), so the v->tau chain is
        # S1,cnt,G,rc,tau.
        nsteps = N0 if t == 0 else 1
        if t == 0:
            # analytic warm start: tau0 = (sum(v) - 1)/P
            e0 = eop("dve", lambda: TS(dumA[:], vrow[:], 0.0, None,
                                       ALU.add, ALU.add,
                                       accum_out=S1a[:]),
                     waits=[("dve", max(E_v, E_tau))])
            e0 = eop("dve", lambda: TS(tau[:], S1a[:], 1.0 / P,
                                       -1.0 / P, ALU.mult, ALU.add),
                     waits=[("dve", e0)])
            E_tau = eop("dve", lambda: TS(X64[:], tau[:], -float(P),
                                          -1.0, ALU.mult, ALU.add),
                        waits=[("dve", e0)])
        for k in range(nsteps):
            e1 = eop("dve", lambda: TS(dumA[:], vrow[:], tau[:], None,
                                       ALU.max, ALU.add, accum_out=S1a[:]),
                     waits=[("dve", max(E_v, E_tau))])
            e2 = eop("dve", lambda: TS(dumB[:], vrow[:], tau[:], None,
                                       ALU.is_gt, ALU.add,
                                       accum_out=cnta[:]),
                     waits=[("dve", e1)])
            e = eop("dve", lambda: TT(G64[:], X64[:], S1a[:], ALU.add),
                    waits=[("dve", e2)])
            e = eop("dve", lambda: nc.vector.reciprocal(rc[:], cnta[:]),
                    waits=[("dve", e)])
            E_tau = eop("dve", lambda: STT(tau[:], G64[:], rc[:], tau[:],
                                           ALU.mult, ALU.add),
                        waits=[("dve", e)])
            if k < nsteps - 1:
                E_tau = eop("dve", lambda: TS(X64[:], tau[:], -float(P),
                                              -1.0, ALU.mult, ALU.add),
                            waits=[("dve", E_tau)])

        # w = clip(v - tau, 0, MAX_W)
        e = eop("dve", lambda: TS(t1[:], vrow[:], tau[:], 0.0,
                                  ALU.subtract, ALU.max),
                waits=[("dve", E_tau)])
        E_wn = eop("dve", lambda: TS(wnew[:], t1[:], MAX_W, None, ALU.min),
                   waits=[("dve", e)])

        if not last:
            E_zh0 = 0
            E_z = eop("dve", lambda: STT(zrow[:], wnew[:], TH1, ws[:],
                                         ALU.mult, ALU.subtract),
                      waits=[("dve", max(E_wn, E_ws))])
            E_zh0 = E_z
            E_ws = eop("dve", lambda: TS(ws[:], wnew[:], TH, None,
                                        ALU.mult), waits=[("dve", E_z)])
            eop("dve", lambda: TS(X64[:], tau[:], -float(P), -1.0,
                                  ALU.mult, ALU.add),
                waits=[("dve", E_ws)])
        else:
            # renormalize: out = w / (sum(w) + EPS)
            e = eop("dve", lambda: TS(dumA[:], wnew[:], 0.0, None,
                                      ALU.add, ALU.add,
                                      accum_out=ssum[:]),
                    waits=[("dve", E_wn)])
            e = eop("dve", lambda: TS(ssum[:], ssum[:], EPS, None,
                                      ALU.add), waits=[("dve", e)])
            e = eop("dve", lambda: nc.vector.reciprocal(rs[:], ssum[:]),
                    waits=[("dve", e)])
            E_out = eop("dve", lambda: TS(outt[:], wnew[:], rs[:], None,
                                          ALU.mult), waits=[("dve", e)])

    # ---------------------------------------------------------- store
    nc.sync.wait_ge(sems["dve"], E_out)
    d = nc.sync.dma_start(out=out_d, in_=outt[:])
    d.then_inc(sems["dma_out"], 16)
    nc.sync.wait_ge(sems["dma_out"], 16)


def build(lam1, lam2, T=None, npow=None):
    nc = bass.Bass("TRN2", target_bir_lowering=False, debug=False)
    sigma_d = nc.dram_tensor("sigma", [NB, P, P], F32, kind="ExternalInput")
    beta_d = nc.dram_tensor("beta", [NB, P], F32, kind="ExternalInput")
    wprev_d = nc.dram_tensor("w_prev", [NB, P], F32, kind="ExternalInput")
    out_d = nc.dram_tensor("out", [NB, P], F32, kind="ExternalOutput")
    with ExitStack() as ctx:
        _emit(ctx, nc, sigma_d.ap(), beta_d.ap(), wprev_d.ap(), out_d.ap(),
              lam1, lam2, T=T, npow=npow)
    return nc


def kernel(sigma, beta, w_prev, log_lambda1, log_lambda2):
    global LAST_RESULT
    sigma = np.ascontiguousarray(np.asarray(sigma, dtype=np.float32))
    beta = np.ascontiguousarray(np.asarray(beta, dtype=np.float32))
    w_prev = np.ascontiguousarray(np.asarray(w_prev, dtype=np.float32))
    lam1 = float(np.exp(np.float32(log_lambda1)))
    lam2 = float(np.exp(np.float32(log_lambda2)))

    nc = build(lam1, lam2)
    in_maps = []
    for c in range(N_CORES):
        s = slice(c * NB, (c + 1) * NB)
        in_maps.append({
            "sigma": sigma[s],
            "beta": beta[s],
            "w_prev": w_prev[s],
        })
    res = run_bass_kernel_spmd(nc, in_maps, list(range(N_CORES)), trace=TRACE)
    LAST_RESULT = res
    out = np.concatenate([res.results[c]["out"] for c in range(N_CORES)],
                         axis=0)
    return np.ascontiguousarray(out.astype(np.float32))
